# revision 64
# baseline (speedup 1.0000x reference)
"""Multi-head attention kernel for 8 TRN2 NeuronCores.

Problem: b=2, n=2048, d=1024, heads=16, hd=64.
  q/k/v = x @ W{q,k,v}.T (+ zero bias)
  per head: softmax(q k^T / sqrt(d)) @ v
  out = concat @ Wo.T (+ zero bias)

Sharding (8 cores): data-parallel over batch (2) x tensor-parallel over
heads (16 heads -> 4 groups of 4). Core c handles batch c//4, heads
4*(c%4) .. 4*(c%4)+3 (feature slice of 256 columns). Wo is applied
row-parallel: each core emits a partial output; the host sums the 4
partials per batch (and untransposes). No collectives needed.

v2 structure (measured-v1 post-mortem):
 - everything bf16 on SBUF/DRAM (PSUM accumulation stays f32): halves
   input DMA (startup was DMA-gated ~12us), halves SBUF, keeps the PE
   at the same 1 cyc/row as f32r but without the moving>=256 caveat.
 - attention runs in q-QUARTERS (SCW=512): passes are (quarter, head)
   ordered, so the output projection for quarter qq runs as soon as the
   4 heads of qq are done -- the old version ran ALL of wo_half(1) after
   the last pass, leaving a ~15us PE-only tail at degraded clock.
 - scores psum pool has 3 bufs + pt pool 3 bufs so the scheduler runs
   scores ~2 kc ahead of the ACT exp stream (absorbs exp jitter).
 - normalize chain per pass_end is row-copy(DVE) -> partition_broadcast
   of the raw sums (GpSimd) -> reciprocal on 64 partitions (DVE) ->
   multiply reading avo straight from PSUM. The v1 chain bounced the
   sums through two SBUF DMAs (reciprocal-then-broadcast) which cost
   ~9us of serial latency at the tail.
 - K^T is stored zero-padded per head to a full 128-row stationary
   (K=64 matmuls run at 2 cyc/row on HW; padded K=128 runs at 1).
 - V is built in natural [n, feat] layout with a ones column appended
   (the ones column accumulates softmax denominators during AV).
 - projections stream behind a column-split xT DMA (quarter 0 columns
   of every contraction chunk land first so the first Q/K stages and
   the first pass start ~3us in).

Biases are structurally zero in this problem spec and are skipped.
"""

import numpy as np

HEADS = 16
D = 1024
N = 2048
B = 2
N_CORES = 8
HPC = HEADS // (N_CORES // B)  # heads per core = 4
HD = D // HEADS                # 64
F = HPC * HD                   # 256 features per core
P = 128


def build_nc(n=N, d=D, hpc=HPC, hd=HD):
    """Build the per-core Bass program (SPMD: same program on all 8 cores)."""
    import concourse.bass as bass
    import concourse.tile as tile
    from concourse import bacc, mybir

    f32 = mybir.dt.float32
    bf16 = mybir.dt.bfloat16
    f = hpc * hd            # per-core feature count (256)
    FC = f // P             # feature chunks (2)
    DC = d // P             # contraction chunks over d (8)
    NT = n // P             # n tiles / k chunks (16)
    SCW = 512               # scores width = one q-quarter = one psum bank
    NQ = n // SCW           # q-quarters (4)
    KG = NT // NQ           # k-chunks per xT column sweep (4)
    scale = 1.0 / float(np.sqrt(np.float32(d)))

    nc = bacc.Bacc("TRN2")

    # Inputs are HOST-PREARRANGED into the exact SBUF tile layouts so each
    # weight is ONE dma_start with 4KB-contiguous descriptor rows and xT is
    # one start per column sweep -- per-dma_start descriptor generation on
    # the sync queue (~605ns each) was the startup gate with 34 starts.
    xT = nc.declare_dram_parameter("xT", [NQ, P, DC, SCW], bf16, isOutput=False)
    wqT = nc.declare_dram_parameter("wqT", [P, DC, f], bf16, isOutput=False)
    wkT = nc.declare_dram_parameter("wkT", [P, DC, f], bf16, isOutput=False)
    wvT = nc.declare_dram_parameter("wvT", [P, DC, f], bf16, isOutput=False)
    woT = nc.declare_dram_parameter("woT", [P, FC, d], bf16, isOutput=False)
    out = nc.declare_dram_parameter("out", [d, n], bf16, isOutput=True)
    # fc0 half of quarter 3's output projection -- written during quarter 3
    # and summed with out's fc1-only quarter-3 columns on the host, so the
    # tail never runs a PSUM-accumulate + add chain
    out2 = nc.declare_dram_parameter("out2", [d, 512], bf16, isOutput=True)

    with tile.TileContext(nc) as tc:
        with (
            tc.tile_pool(name="qkv", bufs=1) as qkv,
            tc.tile_pool(name="outT", bufs=1) as outp,
            tc.tile_pool(name="xw", bufs=1) as xw,
            tc.tile_pool(name="pt", bufs=3) as ptp,
            tc.tile_pool(name="ptb", bufs=3) as ptpb,
            tc.tile_pool(name="norm", bufs=2) as normp,
            tc.tile_pool(name="wosb", bufs=4) as wosbp,
            tc.tile_pool(name="scps", bufs=4, space="PSUM") as scps,
            tc.tile_pool(name="avps", bufs=2, space="PSUM") as avps,
            tc.tile_pool(name="gen", bufs=2, space="PSUM") as genp,
        ):
            QT_sb = qkv.tile([P, FC, n], bf16)
            # per-head K^T, zero-padded to a full 128-row stationary (head h
            # occupies partition rows po..po+hd, matching its rows in QT)
            KTz_sb = qkv.tile([P, hpc, n], bf16)
            V_sb = qkv.tile([P, NT, hpc, hd + 2], bf16)
            # one outT tile PER QUARTER: the tile framework tracks deps at
            # tile granularity, so a shared outT would serialize every wo
            # block on the most recent pass_end mul
            outTq = [
                outp.tile([P, FC, SCW], bf16, name=f"outTq{i}")
                for i in range(NQ)
            ]
            # sweep-major xT: slice [:, g, :, :] is contiguous per partition
            # (8KB rows) so one dma_start covers a whole column sweep
            xT_r = xw.tile([P, NQ, DC, SCW], bf16)
            wqT_r = xw.tile([P, DC, f], bf16)
            wkT_r = xw.tile([P, DC, f], bf16)
            wvT_r = xw.tile([P, DC, f], bf16)
            woT_sb = xw.tile([P, FC, d], bf16)

            # ones column of V_aug (accumulates softmax denominators in AV)
            nc.vector.memset(V_sb[:, :, :, hd : hd + 1], 1.0)
            nc.vector.memset(V_sb[:, :, :, hd + 1 : hd + 2], 0.0)

            def ktz_zero(g):
                """Zero the sweep-g columns of every head's padded K^T slab
                (just-in-time: only sweep 0 gates the first pass)."""
                nc.vector.memset(
                    KTz_sb[:, :, g * SCW : (g + 1) * SCW], 0.0
                )

            def xsweep(g):
                """xT column sweep g, split across the sync+gpsimd queues.
                Emitted JUST-IN-TIME before its first consumer: tile deps
                batch coarsely, so a consumer emitted after a dma_start
                waits for every earlier writer of that tile."""
                nc.sync.dma_start(
                    out=xT_r[:, g, 0 : DC // 2, :], in_=xT[g][:, 0 : DC // 2, :]
                )
                nc.gpsimd.dma_start(
                    out=xT_r[:, g, DC // 2 :, :], in_=xT[g][:, DC // 2 :, :]
                )

            # first-use-ordered input stream: weights serial on the scalar
            # queue (idle until the first exp ~9us in), sweeps on sync+gpsimd
            nc.scalar.dma_start(out=wqT_r[:], in_=wqT[:])
            xsweep(0)
            nc.sync.dma_start(out=wkT_r[:, 0 : DC // 2, :], in_=wkT[:, 0 : DC // 2, :])
            nc.gpsimd.dma_start(out=wkT_r[:, DC // 2 :, :], in_=wkT[:, DC // 2 :, :])
            nc.scalar.dma_start(out=wvT_r[:], in_=wvT[:])

            def qs(fc, qq):
                """Q^T projection stage: heads 2fc,2fc+1, q columns of
                quarter qq (dc-outer accumulation, one psum bank)."""
                ps = genp.tile([P, SCW], f32, tag="gen", name=f"q{fc}{qq}")
                for dc in range(DC):
                    nc.tensor.matmul(
                        ps[:],
                        wqT_r[:, dc, fc * P : (fc + 1) * P],
                        xT_r[:, qq, dc, :],
                        start=(dc == 0),
                        stop=(dc == DC - 1),
                    )
                nc.vector.tensor_copy(
                    QT_sb[:, fc, qq * SCW : (qq + 1) * SCW], ps[:]
                )

            def ks(fc, g):
                """K^T projection stage: heads 2fc,2fc+1, k columns of
                sweep g; rows land in each head's padded KTz slot."""
                ps = genp.tile([P, SCW], f32, tag="gen", name=f"k{fc}{g}")
                for dc in range(DC):
                    nc.tensor.matmul(
                        ps[:],
                        wkT_r[:, dc, fc * P : (fc + 1) * P],
                        xT_r[:, g, dc, :],
                        start=(dc == 0),
                        stop=(dc == DC - 1),
                    )
                sl = slice(g * SCW, (g + 1) * SCW)
                nc.vector.tensor_copy(
                    KTz_sb[0:hd, 2 * fc, sl], ps[0:hd, :]
                )
                nc.vector.tensor_copy(
                    KTz_sb[hd : 2 * hd, 2 * fc + 1, sl],
                    ps[hd : 2 * hd, :],
                )

            def v_tile(nt):
                """V tile nt in natural [n, feat] layout (stationary = xT
                chunk, moving = wv)."""
                ps = genp.tile([P, SCW], f32, tag="gen", name=f"v{nt}")
                g, j = nt // KG, nt % KG
                for dc in range(DC):
                    nc.tensor.matmul(
                        ps[:, 0:f],
                        xT_r[:, g, dc, j * P : (j + 1) * P],
                        wvT_r[:, dc, :],
                        start=(dc == 0),
                        stop=(dc == DC - 1),
                    )
                nc.vector.tensor_copy(
                    V_sb[:, nt, :, 0:hd],
                    ps[:, 0:f].rearrange("p (h e) -> p h e", h=hpc),
                )

            def pass_begin():
                # pav holds the deferred AV emitter: AV(kc) is emitted after
                # S(kc+1), one exp-period after its pt was written -- an AV
                # issued the moment its exp's semaphore fires reads pt while
                # the ACT's SBUF writes are still committing (+~130ns, seen
                # on nearly every AV of the projection-free quarters)
                return {
                    "avo": avps.tile([hd + 2, SCW], f32, tag="avo", name="avo"),
                    "pav": None,
                }

            def pass_blocks(pd, h, qq, kcs, pre_kc=None, q0=None, w=SCW):
                """scores^T -> exp -> AV accumulate for k-chunks `kcs`.
                q0/w override the q-column window (the final pass runs as
                two 256-wide sub-passes so its tail overlaps wo)."""
                fc = (h * hd) // P
                if q0 is None:
                    q0 = qq * SCW
                avo = pd["avo"]
                for kc in kcs:
                    if pre_kc is not None:
                        pre_kc(kc)
                    sc = scps.tile([P, SCW], f32, tag="sc")
                    nc.tensor.matmul(
                        sc[:, 0:w],
                        KTz_sb[:, h, kc * P : (kc + 1) * P],
                        QT_sb[:, fc, q0 : q0 + w],
                        start=True,
                        stop=True,
                    )
                    if pd["pav"] is not None:
                        pd["pav"]()
                    pt = (ptp if kc % 2 == 0 else ptpb).tile(
                        [P, SCW], bf16, tag="pt"
                    )
                    nc.scalar.activation(
                        pt[:, 0:w], sc[:, 0:w],
                        mybir.ActivationFunctionType.Exp,
                        scale=scale,
                    )

                    def pav(kc=kc, pt=pt):
                        nc.tensor.matmul(
                            avo[:, 0:w],
                            V_sb[:, kc, h, :],
                            pt[:, 0:w],
                            start=(kc == 0),
                            stop=(kc == NT - 1),
                        )

                    pd["pav"] = pav

            def pass_end(pd, h, qq, q0=None, w=SCW):
                if pd["pav"] is not None:
                    pd["pav"]()
                    pd["pav"] = None
                avo = pd["avo"]
                """Normalize rows 0..hd-1 of avo by row hd (softmax sums):
                approx-reciprocal the PSUM sums row (sums are O(1e3) --
                far from the approximation's edge cases), partition-
                broadcast, multiply straight out of PSUM into outT."""
                fc = (h * hd) // P
                po = (h * hd) % P
                if q0 is None:
                    q0 = qq * SCW
                o0 = q0 - qq * SCW
                sums = normp.tile([1, SCW], f32, tag="sums")
                nc.vector.tensor_copy(
                    sums[:, 0:w], avo[hd : hd + 1, o0 : o0 + w]
                )
                rrow = normp.tile([1, SCW], f32, tag="rrow")
                nc.vector.reciprocal_approx_fast(rrow[:, 0:w], sums[:, 0:w])
                bc = normp.tile([hd, SCW], f32, tag="bc")
                nc.gpsimd.partition_broadcast(bc[:, 0:w], rrow[:, 0:w])
                nc.vector.tensor_mul(
                    outTq[qq][po : po + hd, fc, o0 : o0 + w],
                    avo[0:hd, o0 : o0 + w],
                    bc[:, 0:w],
                )

            # Each pass's normalize chain is EMITTED near the END of the
            # next pass: chains emitted at their natural spot make every
            # later-emitted PE instruction coarse-wait on their mul. The
            # leftover AV flushes early (chunk 0) so its pt buffer frees,
            # while the chain lands before the last 2 kc so only those
            # scores sit behind the (long-since-computed) mul.
            pending_end = [None]

            def flush_av():
                if pending_end[0] is not None:
                    pd = pending_end[0][0]
                    if pd["pav"] is not None:
                        pd["pav"]()
                        pd["pav"] = None

            def flush_end():
                if pending_end[0] is not None:
                    pd, h, qq = pending_end[0]
                    pass_end(pd, h, qq)
                    pending_end[0] = None

            def do_pass(h, qq, pre_kc=None, mid=None):
                avo = pass_begin()
                pass_blocks(avo, h, qq, range(0, KG), pre_kc=pre_kc)
                flush_av()
                pass_blocks(avo, h, qq, range(KG, NT // 2), pre_kc=pre_kc)
                if mid is not None:
                    mid()
                pass_blocks(avo, h, qq, range(NT // 2, NT - 2), pre_kc=pre_kc)
                flush_end()
                pass_blocks(avo, h, qq, range(NT - 2, NT), pre_kc=pre_kc)
                pending_end[0] = (avo, h, qq)

            def wo_blocks(qq, dos, copy_eng="dve", pool=None):
                """Output projection for quarter qq, do-blocks `dos`
                (contract both fc chunks; emits the partial TRANSPOSED
                [d, n]). PSUM->SBUF copies alternate ACT/DVE so neither
                queue backs up ahead of the next quarter's exp/AV chain;
                copy_eng="act" keeps the DVE free (tail filler blocks run
                during the final normalize chain, which lives on DVE)."""
                q0 = qq * SCW
                for do in dos:
                    pl = pool if pool is not None else genp
                    tg = "sc" if pool is not None else "gen"
                    ps = pl.tile([P, SCW], f32, tag=tg, name=f"wo{do}")
                    for fc in range(FC):
                        nc.tensor.matmul(
                            ps[:],
                            woT_sb[:, fc, do * P : (do + 1) * P],
                            outTq[qq][:, fc, :],
                            start=(fc == 0),
                            stop=(fc == FC - 1),
                        )
                    ob = wosbp.tile([P, SCW], bf16, tag="ob")
                    if copy_eng == "act" or (copy_eng == "alt" and do % 2 == 0):
                        nc.scalar.activation(
                            ob[:], ps[:], mybir.ActivationFunctionType.Copy
                        )
                    else:
                        nc.vector.tensor_copy(ob[:], ps[:])
                    # sync-queue descriptors process ~4x faster than
                    # gpsimd-queue ones (42ns vs 155ns each, measured), and
                    # the input stream is done by the time wo runs
                    nc.sync.dma_start(
                        out=out[do * P : (do + 1) * P, q0 : q0 + SCW],
                        in_=ob[:],
                    )

            def wo_q3_fc0(dos):
                """The fc0 (heads 0,1) half of quarter 3's output
                projection, emitted inside pass(2,3) as soon as those heads
                are final. Lands in out2, summed on the host."""
                for do in dos:
                    ps = genp.tile([P, SCW], f32, tag="gen", name=f"w3a{do}")
                    nc.tensor.matmul(
                        ps[:],
                        woT_sb[:, 0, do * P : (do + 1) * P],
                        outTq[3][:, 0, :],
                        start=True,
                        stop=True,
                    )
                    ob = wosbp.tile([P, SCW], bf16, tag="ob")
                    nc.vector.tensor_copy(ob[:], ps[:])
                    nc.sync.dma_start(
                        out=out2[do * P : (do + 1) * P, :], in_=ob[:]
                    )

            # persistent tail output staging: fc1 halves land per-do, one
            # full-width DMA per do after its second half
            obq3 = outp.tile([P, d // P, SCW], bf16)

            def wo_q3_fc1(half, dos):
                """Tail: fc1-only wo for one 256-wide half of quarter 3
                (half A runs while half B's normalize chain is still in
                flight). Copies stay on ACT for half A (DVE owns the norm
                chains), alternate for half B."""
                q3 = 3 * SCW
                o0 = half * 256
                for do in dos:
                    ps = scps.tile([P, SCW], f32, tag="sc", name=f"w3b{do}")
                    nc.tensor.matmul(
                        ps[:, 0:256],
                        woT_sb[:, 1, do * P : (do + 1) * P],
                        outTq[3][:, 1, o0 : o0 + 256],
                        start=True,
                        stop=True,
                    )
                    if half == 0:
                        nc.scalar.activation(
                            obq3[:, do, o0 : o0 + 256], ps[:, 0:256],
                            mybir.ActivationFunctionType.Copy,
                        )
                    else:
                        nc.vector.tensor_copy(
                            obq3[:, do, o0 : o0 + 256], ps[:, 0:256]
                        )
                    if half == 1:
                        eng = nc.sync if do % 2 == 0 else nc.gpsimd
                        eng.dma_start(
                            out=out[do * P : (do + 1) * P, q3 : q3 + SCW],
                            in_=obq3[:, do, :],
                        )

            # ---- emission order = scheduling priority ----
            # quarter 0: the xT DMA stream is the gate; interleave the h0
            # and h1 passes sweep-by-sweep so every landed sweep unlocks
            # ~2x the PE work (both heads' scores + the fc1 projections
            # that only need sweep 0)
            ktz_zero(0)
            qs(0, 0)
            ks(0, 0)
            avo0 = pass_begin()
            pass_blocks(avo0, 0, 0, range(0, KG), pre_kc=v_tile)
            ks(1, 0)
            qs(1, 0)
            avo1 = pass_begin()
            pass_blocks(avo1, 1, 0, range(0, KG))
            xsweep(1)
            ktz_zero(1)
            ks(0, 1)
            pass_blocks(avo0, 0, 0, range(KG, 2 * KG), pre_kc=v_tile)
            ks(1, 1)
            pass_blocks(avo1, 1, 0, range(KG, 2 * KG))
            xsweep(2)
            ktz_zero(2)
            ks(0, 2)
            pass_blocks(avo0, 0, 0, range(2 * KG, 3 * KG), pre_kc=v_tile)
            ks(1, 2)
            pass_blocks(avo1, 1, 0, range(2 * KG, 3 * KG))
            xsweep(3)
            ktz_zero(3)
            ks(0, 3)
            pass_blocks(avo0, 0, 0, range(3 * KG, NT), pre_kc=v_tile)
            ks(1, 3)
            nc.sync.dma_start(out=woT_sb[:], in_=woT[:])
            pass_blocks(avo1, 1, 0, range(3 * KG, NT))
            pass_end(avo0, 0, 0)
            pending_end[0] = (avo1, 1, 0)

            do_pass(2, 0, mid=lambda: qs(0, 1))
            do_pass(3, 0, mid=lambda: qs(1, 1))
            # wo for a finished quarter is spread through the next quarter's
            # passes via the mid hook: cross-engine waits batch coarsely
            # (anything emitted after a pass_end waits on its mul), so the
            # blocks must be emitted BEFORE the surrounding pass_end
            def mids(*fns):
                return lambda: [fn() for fn in fns]

            do_pass(0, 1, mid=lambda: qs(0, 2))
            do_pass(1, 1, mid=lambda: qs(1, 2))
            do_pass(2, 1, mid=lambda: qs(0, 3))
            do_pass(3, 1, mid=lambda: qs(1, 3))
            for h in range(hpc):
                do_pass(h, 2, mid=lambda h=h: wo_blocks(0, [2 * h, 2 * h + 1]))
            do_pass(0, 3, mid=lambda: wo_blocks(1, [0, 1, 2, 3]))
            do_pass(1, 3, mid=lambda: wo_blocks(1, [4, 5, 6, 7]))
            # fc0's out2 stream rides pass(2,3): heads 0,1 are done, and
            # emitting it here lets its 1MB of output DMA drain during
            # compute instead of stacking onto the post-kernel drain
            do_pass(
                2, 3,
                mid=mids(
                    lambda: wo_blocks(2, [0, 1, 2, 3]),
                    lambda: wo_q3_fc0(range(8)),
                ),
            )
            # final pass. The tail is software-pipelined: the normalize is
            # split into two half-width chains, fc0 filler blocks keep the
            # PE warm through chain A, fc1's half-A wo overlaps chain B.
            q3 = 3 * SCW
            avoz = pass_begin()
            pass_blocks(avoz, 3, 3, range(0, KG))
            flush_av()
            pass_blocks(avoz, 3, 3, range(KG, NT // 2))
            wo_blocks(2, [4, 5], copy_eng="dve")
            pass_blocks(avoz, 3, 3, range(NT // 2, NT - 2))
            flush_end()
            pass_blocks(avoz, 3, 3, range(NT - 2, NT))
            wo_blocks(2, [6, 7], copy_eng="dve")
            pass_end(avoz, 3, 3, q0=q3, w=256)
            wo_q3_fc1(0, range(8))
            pass_end(avoz, 3, 3, q0=q3 + 256, w=256)
            wo_q3_fc1(1, range(8))
    nc.finalize()
    return nc


def make_in_maps(x, Wq, Wk, Wv, Wo):
    """Shard full inputs into per-core DRAM parameter maps (bf16)."""
    import ml_dtypes

    bf16 = ml_dtypes.bfloat16
    DC, NQ, SCW = D // P, N // 512, 512

    def w_pre(wT):  # [d_or_f, cols] -> [P, chunks, cols]
        return np.ascontiguousarray(
            wT.reshape(-1, P, wT.shape[1]).transpose(1, 0, 2)
        ).astype(bf16)

    x = np.asarray(x, dtype=np.float32)
    # [d, n] -> sweep-major [NQ, P, DC, SCW] matching the xT_r tile
    xTs = [
        np.ascontiguousarray(
            x[b].T.reshape(DC, P, NQ, SCW).transpose(2, 1, 0, 3)
        ).astype(bf16)
        for b in range(B)
    ]
    WqT = np.asarray(Wq, np.float32).T
    WkT = np.asarray(Wk, np.float32).T
    WvT = np.asarray(Wv, np.float32).T
    WoB = np.asarray(Wo, np.float32)
    in_maps = []
    for c in range(N_CORES):
        b, g = c // (N_CORES // B), c % (N_CORES // B)
        fs = slice(g * F, (g + 1) * F)
        in_maps.append(
            {
                "xT": xTs[b],
                "wqT": w_pre(WqT[:, fs]),
                "wkT": w_pre(WkT[:, fs]),
                "wvT": w_pre(WvT[:, fs]),
                "woT": w_pre(np.ascontiguousarray(WoB[:, fs].T)),
            }
        )
    return in_maps


_NC_CACHE = {}


def run(x, Wq, Wk, Wv, Wo, trace=False):
    from concourse.bass_utils import run_bass_kernel_spmd

    # NOTE: walrus --enable-ldw-opt stays at its default (false): v2 has no
    # consecutive matmuls sharing a stationary (SCW == QB == 512), and the
    # bf16 Ldweights form is rejected by the opt's codegen path anyway.
    if "nc" not in _NC_CACHE:
        _NC_CACHE["nc"] = build_nc()
    nc = _NC_CACHE["nc"]
    in_maps = make_in_maps(x, Wq, Wk, Wv, Wo)
    res = run_bass_kernel_spmd(nc, in_maps, core_ids=list(range(N_CORES)), trace=trace)
    parts = []
    for i in range(N_CORES):
        p = np.asarray(res.results[i]["out"]).astype(np.float32)
        # quarter 3's output was emitted in two halves: fc1 went to out,
        # fc0 to out2 -- recombine here
        p[:, 3 * 512 :] += np.asarray(res.results[i]["out2"]).astype(
            np.float32
        )
        parts.append(p)
    gpb = N_CORES // B
    # per-core partials are transposed [d, n]: sum the group, then untranspose
    full = np.stack(
        [
            sum(parts[b * gpb + 1 : (b + 1) * gpb], parts[b * gpb]).T
            for b in range(B)
        ]
    )
    return np.ascontiguousarray(full, dtype=np.float32), res


def kernel(x, Wq, bq, Wk, bk, Wv, bv, Wo, bo):
    full, _ = run(x, Wq, Wk, Wv, Wo)
    return full


# revision 65
# speedup vs baseline: 1.0200x; 1.0200x over previous
"""Multi-head attention kernel for 8 TRN2 NeuronCores.

Problem: b=2, n=2048, d=1024, heads=16, hd=64.
  q/k/v = x @ W{q,k,v}.T (+ zero bias)
  per head: softmax(q k^T / sqrt(d)) @ v
  out = concat @ Wo.T (+ zero bias)

Sharding (8 cores): data-parallel over batch (2) x tensor-parallel over
heads (16 heads -> 4 groups of 4). Core c handles batch c//4, heads
4*(c%4) .. 4*(c%4)+3 (feature slice of 256 columns). Wo is applied
row-parallel: each core emits a partial output; the host sums the 4
partials per batch (and untransposes). No collectives needed.

v2 structure (measured-v1 post-mortem):
 - everything bf16 on SBUF/DRAM (PSUM accumulation stays f32): halves
   input DMA (startup was DMA-gated ~12us), halves SBUF, keeps the PE
   at the same 1 cyc/row as f32r but without the moving>=256 caveat.
 - attention runs in q-QUARTERS (SCW=512): passes are (quarter, head)
   ordered, so the output projection for quarter qq runs as soon as the
   4 heads of qq are done -- the old version ran ALL of wo_half(1) after
   the last pass, leaving a ~15us PE-only tail at degraded clock.
 - scores psum pool has 3 bufs + pt pool 3 bufs so the scheduler runs
   scores ~2 kc ahead of the ACT exp stream (absorbs exp jitter).
 - normalize chain per pass_end is row-copy(DVE) -> partition_broadcast
   of the raw sums (GpSimd) -> reciprocal on 64 partitions (DVE) ->
   multiply reading avo straight from PSUM. The v1 chain bounced the
   sums through two SBUF DMAs (reciprocal-then-broadcast) which cost
   ~9us of serial latency at the tail.
 - K^T is stored zero-padded per head to a full 128-row stationary
   (K=64 matmuls run at 2 cyc/row on HW; padded K=128 runs at 1).
 - V is built in natural [n, feat] layout with a ones column appended
   (the ones column accumulates softmax denominators during AV).
 - projections stream behind a column-split xT DMA (quarter 0 columns
   of every contraction chunk land first so the first Q/K stages and
   the first pass start ~3us in).

Biases are structurally zero in this problem spec and are skipped.
"""

import numpy as np

HEADS = 16
D = 1024
N = 2048
B = 2
N_CORES = 8
HPC = HEADS // (N_CORES // B)  # heads per core = 4
HD = D // HEADS                # 64
F = HPC * HD                   # 256 features per core
P = 128


def build_nc(n=N, d=D, hpc=HPC, hd=HD):
    """Build the per-core Bass program (SPMD: same program on all 8 cores)."""
    import concourse.bass as bass
    import concourse.tile as tile
    from concourse import bacc, mybir

    f32 = mybir.dt.float32
    bf16 = mybir.dt.bfloat16
    f = hpc * hd            # per-core feature count (256)
    FC = f // P             # feature chunks (2)
    DC = d // P             # contraction chunks over d (8)
    NT = n // P             # n tiles / k chunks (16)
    SCW = 512               # scores width = one q-quarter = one psum bank
    NQ = n // SCW           # q-quarters (4)
    KG = NT // NQ           # k-chunks per xT column sweep (4)
    scale = 1.0 / float(np.sqrt(np.float32(d)))

    nc = bacc.Bacc("TRN2")

    # Inputs are HOST-PREARRANGED into the exact SBUF tile layouts so each
    # weight is ONE dma_start with 4KB-contiguous descriptor rows and xT is
    # one start per column sweep -- per-dma_start descriptor generation on
    # the sync queue (~605ns each) was the startup gate with 34 starts.
    xT = nc.declare_dram_parameter("xT", [NQ, P, DC, SCW], bf16, isOutput=False)
    wqT = nc.declare_dram_parameter("wqT", [P, DC, f], bf16, isOutput=False)
    wkT = nc.declare_dram_parameter("wkT", [P, DC, f], bf16, isOutput=False)
    wvT = nc.declare_dram_parameter("wvT", [P, DC, f], bf16, isOutput=False)
    woT = nc.declare_dram_parameter("woT", [P, FC, d], bf16, isOutput=False)
    out = nc.declare_dram_parameter("out", [d, n], bf16, isOutput=True)
    # fc0 half of quarter 3's output projection -- written during quarter 3
    # and summed with out's fc1-only quarter-3 columns on the host, so the
    # tail never runs a PSUM-accumulate + add chain
    out2 = nc.declare_dram_parameter("out2", [d, 512], bf16, isOutput=True)

    with tile.TileContext(nc) as tc:
        with (
            tc.tile_pool(name="qkv", bufs=1) as qkv,
            tc.tile_pool(name="outT", bufs=1) as outp,
            tc.tile_pool(name="xw", bufs=1) as xw,
            tc.tile_pool(name="pt", bufs=3) as ptp,
            tc.tile_pool(name="ptb", bufs=3) as ptpb,
            tc.tile_pool(name="norm", bufs=2) as normp,
            tc.tile_pool(name="wosb", bufs=4) as wosbp,
            tc.tile_pool(name="scps", bufs=2, space="PSUM") as scps,
            tc.tile_pool(name="avps", bufs=2, space="PSUM") as avps,
            tc.tile_pool(name="gen", bufs=2, space="PSUM") as genp,
        ):
            QT_sb = qkv.tile([P, FC, n], bf16)
            # per-head K^T, zero-padded to a full 128-row stationary (head h
            # occupies partition rows po..po+hd, matching its rows in QT)
            KTz_sb = qkv.tile([P, hpc, n], bf16)
            V_sb = qkv.tile([P, NT, hpc, hd + 2], bf16)
            # one outT tile PER QUARTER: the tile framework tracks deps at
            # tile granularity, so a shared outT would serialize every wo
            # block on the most recent pass_end mul
            outTq = [
                outp.tile([P, FC, SCW], bf16, name=f"outTq{i}")
                for i in range(NQ)
            ]
            # sweep-major xT: slice [:, g, :, :] is contiguous per partition
            # (8KB rows) so one dma_start covers a whole column sweep
            xT_r = xw.tile([P, NQ, DC, SCW], bf16)
            wqT_r = xw.tile([P, DC, f], bf16)
            wkT_r = xw.tile([P, DC, f], bf16)
            wvT_r = xw.tile([P, DC, f], bf16)
            woT_sb = xw.tile([P, FC, d], bf16)

            # ones column of V_aug (accumulates softmax denominators in AV)
            nc.vector.memset(V_sb[:, :, :, hd : hd + 1], 1.0)
            nc.vector.memset(V_sb[:, :, :, hd + 1 : hd + 2], 0.0)

            def ktz_zero(g):
                """Zero the sweep-g columns of every head's padded K^T slab
                (just-in-time: only sweep 0 gates the first pass)."""
                nc.vector.memset(
                    KTz_sb[:, :, g * SCW : (g + 1) * SCW], 0.0
                )

            def xsweep(g):
                """xT column sweep g, split across the sync+gpsimd queues.
                Emitted JUST-IN-TIME before its first consumer: tile deps
                batch coarsely, so a consumer emitted after a dma_start
                waits for every earlier writer of that tile."""
                nc.sync.dma_start(
                    out=xT_r[:, g, 0 : DC // 2, :], in_=xT[g][:, 0 : DC // 2, :]
                )
                nc.gpsimd.dma_start(
                    out=xT_r[:, g, DC // 2 :, :], in_=xT[g][:, DC // 2 :, :]
                )

            # first-use-ordered input stream: weights serial on the scalar
            # queue (idle until the first exp ~9us in), sweeps on sync+gpsimd
            nc.scalar.dma_start(out=wqT_r[:], in_=wqT[:])
            xsweep(0)
            nc.sync.dma_start(out=wkT_r[:, 0 : DC // 2, :], in_=wkT[:, 0 : DC // 2, :])
            nc.gpsimd.dma_start(out=wkT_r[:, DC // 2 :, :], in_=wkT[:, DC // 2 :, :])
            nc.scalar.dma_start(out=wvT_r[:], in_=wvT[:])

            def qs(fc, qq):
                """Q^T projection stage: heads 2fc,2fc+1, q columns of
                quarter qq (dc-outer accumulation, one psum bank)."""
                ps = genp.tile([P, SCW], f32, tag="gen", name=f"q{fc}{qq}")
                for dc in range(DC):
                    nc.tensor.matmul(
                        ps[:],
                        wqT_r[:, dc, fc * P : (fc + 1) * P],
                        xT_r[:, qq, dc, :],
                        start=(dc == 0),
                        stop=(dc == DC - 1),
                    )
                nc.vector.tensor_copy(
                    QT_sb[:, fc, qq * SCW : (qq + 1) * SCW], ps[:]
                )

            def ks(fc, g):
                """K^T projection stage: heads 2fc,2fc+1, k columns of
                sweep g; rows land in each head's padded KTz slot."""
                ps = genp.tile([P, SCW], f32, tag="gen", name=f"k{fc}{g}")
                for dc in range(DC):
                    nc.tensor.matmul(
                        ps[:],
                        wkT_r[:, dc, fc * P : (fc + 1) * P],
                        xT_r[:, g, dc, :],
                        start=(dc == 0),
                        stop=(dc == DC - 1),
                    )
                sl = slice(g * SCW, (g + 1) * SCW)
                nc.vector.tensor_copy(
                    KTz_sb[0:hd, 2 * fc, sl], ps[0:hd, :]
                )
                nc.vector.tensor_copy(
                    KTz_sb[hd : 2 * hd, 2 * fc + 1, sl],
                    ps[hd : 2 * hd, :],
                )

            def v_tile(nt):
                """V tile nt in natural [n, feat] layout (stationary = xT
                chunk, moving = wv)."""
                ps = genp.tile([P, SCW], f32, tag="gen", name=f"v{nt}")
                g, j = nt // KG, nt % KG
                for dc in range(DC):
                    nc.tensor.matmul(
                        ps[:, 0:f],
                        xT_r[:, g, dc, j * P : (j + 1) * P],
                        wvT_r[:, dc, :],
                        start=(dc == 0),
                        stop=(dc == DC - 1),
                    )
                nc.vector.tensor_copy(
                    V_sb[:, nt, :, 0:hd],
                    ps[:, 0:f].rearrange("p (h e) -> p h e", h=hpc),
                )

            def pass_begin():
                # pav holds the deferred AV emitter: AV(kc) is emitted after
                # S(kc+1), one exp-period after its pt was written -- an AV
                # issued the moment its exp's semaphore fires reads pt while
                # the ACT's SBUF writes are still committing (+~130ns, seen
                # on nearly every AV of the projection-free quarters)
                return {
                    "avo": avps.tile([hd + 2, SCW], f32, tag="avo", name="avo"),
                    "pav": None,
                }

            def pass_blocks(pd, h, qq, kcs, pre_kc=None, q0=None, w=SCW):
                """scores^T -> exp -> AV accumulate for k-chunks `kcs`.
                q0/w override the q-column window (the final pass runs as
                two 256-wide sub-passes so its tail overlaps wo)."""
                fc = (h * hd) // P
                if q0 is None:
                    q0 = qq * SCW
                avo = pd["avo"]
                kcs = list(kcs)
                # kc PAIRS: one [128, 2, SCW] exp per pair (reads 2 psum
                # banks) halves the ACT instruction count -- ACT is the
                # pacer in the projection-free quarters
                for i in range(0, len(kcs), 2):
                    k0, k1 = kcs[i], kcs[i + 1]
                    sc = scps.tile([P, 2, SCW], f32, tag="sc")
                    for j, kc in ((0, k0), (1, k1)):
                        if pre_kc is not None:
                            pre_kc(kc)
                        nc.tensor.matmul(
                            sc[:, j, 0:w],
                            KTz_sb[:, h, kc * P : (kc + 1) * P],
                            QT_sb[:, fc, q0 : q0 + w],
                            start=True,
                            stop=True,
                        )
                    if pd["pav"] is not None:
                        pd["pav"]()
                    pt = (ptp if k0 % 4 == 0 else ptpb).tile(
                        [P, 2, SCW], bf16, tag="pt"
                    )
                    nc.scalar.activation(
                        pt[:, :, 0:w], sc[:, :, 0:w],
                        mybir.ActivationFunctionType.Exp,
                        scale=scale,
                    )

                    def pav(k0=k0, k1=k1, pt=pt):
                        for j, kc in ((0, k0), (1, k1)):
                            nc.tensor.matmul(
                                avo[:, 0:w],
                                V_sb[:, kc, h, :],
                                pt[:, j, 0:w],
                                start=(kc == 0),
                                stop=(kc == NT - 1),
                            )

                    pd["pav"] = pav

            def pass_end(pd, h, qq, q0=None, w=SCW):
                if pd["pav"] is not None:
                    pd["pav"]()
                    pd["pav"] = None
                avo = pd["avo"]
                """Normalize rows 0..hd-1 of avo by row hd (softmax sums):
                approx-reciprocal the PSUM sums row (sums are O(1e3) --
                far from the approximation's edge cases), partition-
                broadcast, multiply straight out of PSUM into outT."""
                fc = (h * hd) // P
                po = (h * hd) % P
                if q0 is None:
                    q0 = qq * SCW
                o0 = q0 - qq * SCW
                sums = normp.tile([1, SCW], f32, tag="sums")
                nc.vector.tensor_copy(
                    sums[:, 0:w], avo[hd : hd + 1, o0 : o0 + w]
                )
                rrow = normp.tile([1, SCW], f32, tag="rrow")
                nc.vector.reciprocal_approx_fast(rrow[:, 0:w], sums[:, 0:w])
                bc = normp.tile([hd, SCW], f32, tag="bc")
                nc.gpsimd.partition_broadcast(bc[:, 0:w], rrow[:, 0:w])
                nc.vector.tensor_mul(
                    outTq[qq][po : po + hd, fc, o0 : o0 + w],
                    avo[0:hd, o0 : o0 + w],
                    bc[:, 0:w],
                )

            # Each pass's normalize chain is EMITTED near the END of the
            # next pass: chains emitted at their natural spot make every
            # later-emitted PE instruction coarse-wait on their mul. The
            # leftover AV flushes early (chunk 0) so its pt buffer frees,
            # while the chain lands before the last 2 kc so only those
            # scores sit behind the (long-since-computed) mul.
            pending_end = [None]

            def flush_av():
                if pending_end[0] is not None:
                    pd = pending_end[0][0]
                    if pd["pav"] is not None:
                        pd["pav"]()
                        pd["pav"] = None

            def flush_end():
                if pending_end[0] is not None:
                    pd, h, qq = pending_end[0]
                    pass_end(pd, h, qq)
                    pending_end[0] = None

            def do_pass(h, qq, pre_kc=None, mid=None):
                avo = pass_begin()
                pass_blocks(avo, h, qq, range(0, KG), pre_kc=pre_kc)
                flush_av()
                pass_blocks(avo, h, qq, range(KG, NT // 2), pre_kc=pre_kc)
                if mid is not None:
                    mid()
                pass_blocks(avo, h, qq, range(NT // 2, NT - 2), pre_kc=pre_kc)
                flush_end()
                pass_blocks(avo, h, qq, range(NT - 2, NT), pre_kc=pre_kc)
                pending_end[0] = (avo, h, qq)

            def wo_blocks(qq, dos, copy_eng="dve", pool=None):
                """Output projection for quarter qq, do-blocks `dos`
                (contract both fc chunks; emits the partial TRANSPOSED
                [d, n]). PSUM->SBUF copies alternate ACT/DVE so neither
                queue backs up ahead of the next quarter's exp/AV chain;
                copy_eng="act" keeps the DVE free (tail filler blocks run
                during the final normalize chain, which lives on DVE)."""
                q0 = qq * SCW
                for do in dos:
                    pl = pool if pool is not None else genp
                    tg = "sc" if pool is not None else "gen"
                    ps = pl.tile([P, SCW], f32, tag=tg, name=f"wo{do}")
                    for fc in range(FC):
                        nc.tensor.matmul(
                            ps[:],
                            woT_sb[:, fc, do * P : (do + 1) * P],
                            outTq[qq][:, fc, :],
                            start=(fc == 0),
                            stop=(fc == FC - 1),
                        )
                    ob = wosbp.tile([P, SCW], bf16, tag="ob")
                    if copy_eng == "act" or (copy_eng == "alt" and do % 2 == 0):
                        nc.scalar.activation(
                            ob[:], ps[:], mybir.ActivationFunctionType.Copy
                        )
                    else:
                        nc.vector.tensor_copy(ob[:], ps[:])
                    # sync-queue descriptors process ~4x faster than
                    # gpsimd-queue ones (42ns vs 155ns each, measured), and
                    # the input stream is done by the time wo runs
                    nc.sync.dma_start(
                        out=out[do * P : (do + 1) * P, q0 : q0 + SCW],
                        in_=ob[:],
                    )

            def wo_q3_fc0(dos):
                """The fc0 (heads 0,1) half of quarter 3's output
                projection, emitted inside pass(2,3) as soon as those heads
                are final. Lands in out2, summed on the host."""
                for do in dos:
                    ps = genp.tile([P, SCW], f32, tag="gen", name=f"w3a{do}")
                    nc.tensor.matmul(
                        ps[:],
                        woT_sb[:, 0, do * P : (do + 1) * P],
                        outTq[3][:, 0, :],
                        start=True,
                        stop=True,
                    )
                    ob = wosbp.tile([P, SCW], bf16, tag="ob")
                    nc.vector.tensor_copy(ob[:], ps[:])
                    nc.sync.dma_start(
                        out=out2[do * P : (do + 1) * P, :], in_=ob[:]
                    )

            # persistent tail output staging: fc1 halves land per-do, one
            # full-width DMA per do after its second half
            obq3 = outp.tile([P, d // P, SCW], bf16)

            def wo_q3_fc1(half, dos):
                """Tail: fc1-only wo for one 256-wide half of quarter 3
                (half A runs while half B's normalize chain is still in
                flight). Copies stay on ACT for half A (DVE owns the norm
                chains), alternate for half B."""
                q3 = 3 * SCW
                o0 = half * 256
                for do in dos:
                    ps = genp.tile([P, SCW], f32, tag="gen", name=f"w3b{do}")
                    nc.tensor.matmul(
                        ps[:, 0:256],
                        woT_sb[:, 1, do * P : (do + 1) * P],
                        outTq[3][:, 1, o0 : o0 + 256],
                        start=True,
                        stop=True,
                    )
                    if half == 0:
                        nc.scalar.activation(
                            obq3[:, do, o0 : o0 + 256], ps[:, 0:256],
                            mybir.ActivationFunctionType.Copy,
                        )
                    else:
                        nc.vector.tensor_copy(
                            obq3[:, do, o0 : o0 + 256], ps[:, 0:256]
                        )
                    if half == 1:
                        eng = nc.sync if do % 2 == 0 else nc.gpsimd
                        eng.dma_start(
                            out=out[do * P : (do + 1) * P, q3 : q3 + SCW],
                            in_=obq3[:, do, :],
                        )

            # ---- emission order = scheduling priority ----
            # quarter 0: the xT DMA stream is the gate; interleave the h0
            # and h1 passes sweep-by-sweep so every landed sweep unlocks
            # ~2x the PE work (both heads' scores + the fc1 projections
            # that only need sweep 0)
            ktz_zero(0)
            qs(0, 0)
            ks(0, 0)
            avo0 = pass_begin()
            pass_blocks(avo0, 0, 0, range(0, KG), pre_kc=v_tile)
            ks(1, 0)
            qs(1, 0)
            avo1 = pass_begin()
            pass_blocks(avo1, 1, 0, range(0, KG))
            xsweep(1)
            ktz_zero(1)
            ks(0, 1)
            pass_blocks(avo0, 0, 0, range(KG, 2 * KG), pre_kc=v_tile)
            ks(1, 1)
            pass_blocks(avo1, 1, 0, range(KG, 2 * KG))
            xsweep(2)
            ktz_zero(2)
            ks(0, 2)
            pass_blocks(avo0, 0, 0, range(2 * KG, 3 * KG), pre_kc=v_tile)
            ks(1, 2)
            pass_blocks(avo1, 1, 0, range(2 * KG, 3 * KG))
            xsweep(3)
            ktz_zero(3)
            ks(0, 3)
            pass_blocks(avo0, 0, 0, range(3 * KG, NT), pre_kc=v_tile)
            ks(1, 3)
            nc.sync.dma_start(out=woT_sb[:], in_=woT[:])
            pass_blocks(avo1, 1, 0, range(3 * KG, NT))
            pass_end(avo0, 0, 0)
            pending_end[0] = (avo1, 1, 0)

            do_pass(2, 0, mid=lambda: qs(0, 1))
            do_pass(3, 0, mid=lambda: qs(1, 1))
            # wo for a finished quarter is spread through the next quarter's
            # passes via the mid hook: cross-engine waits batch coarsely
            # (anything emitted after a pass_end waits on its mul), so the
            # blocks must be emitted BEFORE the surrounding pass_end
            def mids(*fns):
                return lambda: [fn() for fn in fns]

            do_pass(0, 1, mid=lambda: qs(0, 2))
            do_pass(1, 1, mid=lambda: qs(1, 2))
            do_pass(2, 1, mid=lambda: qs(0, 3))
            do_pass(3, 1, mid=lambda: qs(1, 3))
            for h in range(hpc):
                do_pass(h, 2, mid=lambda h=h: wo_blocks(0, [2 * h, 2 * h + 1]))
            do_pass(0, 3, mid=lambda: wo_blocks(1, [0, 1, 2, 3]))
            do_pass(1, 3, mid=lambda: wo_blocks(1, [4, 5, 6, 7]))
            # fc0's out2 stream rides pass(2,3): heads 0,1 are done, and
            # emitting it here lets its 1MB of output DMA drain during
            # compute instead of stacking onto the post-kernel drain
            do_pass(
                2, 3,
                mid=mids(
                    lambda: wo_blocks(2, [0, 1, 2, 3]),
                    lambda: wo_q3_fc0(range(8)),
                ),
            )
            # final pass. The tail is software-pipelined: the normalize is
            # split into two half-width chains, fc0 filler blocks keep the
            # PE warm through chain A, fc1's half-A wo overlaps chain B.
            q3 = 3 * SCW
            avoz = pass_begin()
            pass_blocks(avoz, 3, 3, range(0, KG))
            flush_av()
            pass_blocks(avoz, 3, 3, range(KG, NT // 2))
            wo_blocks(2, [4, 5], copy_eng="dve")
            pass_blocks(avoz, 3, 3, range(NT // 2, NT - 2))
            flush_end()
            pass_blocks(avoz, 3, 3, range(NT - 2, NT))
            wo_blocks(2, [6, 7], copy_eng="dve")
            pass_end(avoz, 3, 3, q0=q3, w=256)
            wo_q3_fc1(0, range(8))
            pass_end(avoz, 3, 3, q0=q3 + 256, w=256)
            wo_q3_fc1(1, range(8))
    nc.finalize()
    return nc


def make_in_maps(x, Wq, Wk, Wv, Wo):
    """Shard full inputs into per-core DRAM parameter maps (bf16)."""
    import ml_dtypes

    bf16 = ml_dtypes.bfloat16
    DC, NQ, SCW = D // P, N // 512, 512

    def w_pre(wT):  # [d_or_f, cols] -> [P, chunks, cols]
        return np.ascontiguousarray(
            wT.reshape(-1, P, wT.shape[1]).transpose(1, 0, 2)
        ).astype(bf16)

    x = np.asarray(x, dtype=np.float32)
    # [d, n] -> sweep-major [NQ, P, DC, SCW] matching the xT_r tile
    xTs = [
        np.ascontiguousarray(
            x[b].T.reshape(DC, P, NQ, SCW).transpose(2, 1, 0, 3)
        ).astype(bf16)
        for b in range(B)
    ]
    WqT = np.asarray(Wq, np.float32).T
    WkT = np.asarray(Wk, np.float32).T
    WvT = np.asarray(Wv, np.float32).T
    WoB = np.asarray(Wo, np.float32)
    in_maps = []
    for c in range(N_CORES):
        b, g = c // (N_CORES // B), c % (N_CORES // B)
        fs = slice(g * F, (g + 1) * F)
        in_maps.append(
            {
                "xT": xTs[b],
                "wqT": w_pre(WqT[:, fs]),
                "wkT": w_pre(WkT[:, fs]),
                "wvT": w_pre(WvT[:, fs]),
                "woT": w_pre(np.ascontiguousarray(WoB[:, fs].T)),
            }
        )
    return in_maps


_NC_CACHE = {}


def run(x, Wq, Wk, Wv, Wo, trace=False):
    from concourse.bass_utils import run_bass_kernel_spmd

    # NOTE: walrus --enable-ldw-opt stays at its default (false): v2 has no
    # consecutive matmuls sharing a stationary (SCW == QB == 512), and the
    # bf16 Ldweights form is rejected by the opt's codegen path anyway.
    if "nc" not in _NC_CACHE:
        _NC_CACHE["nc"] = build_nc()
    nc = _NC_CACHE["nc"]
    in_maps = make_in_maps(x, Wq, Wk, Wv, Wo)
    res = run_bass_kernel_spmd(nc, in_maps, core_ids=list(range(N_CORES)), trace=trace)
    parts = []
    for i in range(N_CORES):
        p = np.asarray(res.results[i]["out"]).astype(np.float32)
        # quarter 3's output was emitted in two halves: fc1 went to out,
        # fc0 to out2 -- recombine here
        p[:, 3 * 512 :] += np.asarray(res.results[i]["out2"]).astype(
            np.float32
        )
        parts.append(p)
    gpb = N_CORES // B
    # per-core partials are transposed [d, n]: sum the group, then untranspose
    full = np.stack(
        [
            sum(parts[b * gpb + 1 : (b + 1) * gpb], parts[b * gpb]).T
            for b in range(B)
        ]
    )
    return np.ascontiguousarray(full, dtype=np.float32), res


def kernel(x, Wq, bq, Wk, bk, Wv, bv, Wo, bo):
    full, _ = run(x, Wq, Wk, Wv, Wo)
    return full


# revision 66
# speedup vs baseline: 1.0218x; 1.0018x over previous
"""Multi-head attention kernel for 8 TRN2 NeuronCores.

Problem: b=2, n=2048, d=1024, heads=16, hd=64.
  q/k/v = x @ W{q,k,v}.T (+ zero bias)
  per head: softmax(q k^T / sqrt(d)) @ v
  out = concat @ Wo.T (+ zero bias)

Sharding (8 cores): data-parallel over batch (2) x tensor-parallel over
heads (16 heads -> 4 groups of 4). Core c handles batch c//4, heads
4*(c%4) .. 4*(c%4)+3 (feature slice of 256 columns). Wo is applied
row-parallel: each core emits a partial output; the host sums the 4
partials per batch (and untransposes). No collectives needed.

v2 structure (measured-v1 post-mortem):
 - everything bf16 on SBUF/DRAM (PSUM accumulation stays f32): halves
   input DMA (startup was DMA-gated ~12us), halves SBUF, keeps the PE
   at the same 1 cyc/row as f32r but without the moving>=256 caveat.
 - attention runs in q-QUARTERS (SCW=512): passes are (quarter, head)
   ordered, so the output projection for quarter qq runs as soon as the
   4 heads of qq are done -- the old version ran ALL of wo_half(1) after
   the last pass, leaving a ~15us PE-only tail at degraded clock.
 - scores psum pool has 3 bufs + pt pool 3 bufs so the scheduler runs
   scores ~2 kc ahead of the ACT exp stream (absorbs exp jitter).
 - normalize chain per pass_end is row-copy(DVE) -> partition_broadcast
   of the raw sums (GpSimd) -> reciprocal on 64 partitions (DVE) ->
   multiply reading avo straight from PSUM. The v1 chain bounced the
   sums through two SBUF DMAs (reciprocal-then-broadcast) which cost
   ~9us of serial latency at the tail.
 - K^T is stored zero-padded per head to a full 128-row stationary
   (K=64 matmuls run at 2 cyc/row on HW; padded K=128 runs at 1).
 - V is built in natural [n, feat] layout with a ones column appended
   (the ones column accumulates softmax denominators during AV).
 - projections stream behind a column-split xT DMA (quarter 0 columns
   of every contraction chunk land first so the first Q/K stages and
   the first pass start ~3us in).

Biases are structurally zero in this problem spec and are skipped.
"""

import numpy as np

HEADS = 16
D = 1024
N = 2048
B = 2
N_CORES = 8
HPC = HEADS // (N_CORES // B)  # heads per core = 4
HD = D // HEADS                # 64
F = HPC * HD                   # 256 features per core
P = 128


def build_nc(n=N, d=D, hpc=HPC, hd=HD):
    """Build the per-core Bass program (SPMD: same program on all 8 cores)."""
    import concourse.bass as bass
    import concourse.tile as tile
    from concourse import bacc, mybir

    f32 = mybir.dt.float32
    bf16 = mybir.dt.bfloat16
    f = hpc * hd            # per-core feature count (256)
    FC = f // P             # feature chunks (2)
    DC = d // P             # contraction chunks over d (8)
    NT = n // P             # n tiles / k chunks (16)
    SCW = 512               # scores width = one q-quarter = one psum bank
    NQ = n // SCW           # q-quarters (4)
    KG = NT // NQ           # k-chunks per xT column sweep (4)
    scale = 1.0 / float(np.sqrt(np.float32(d)))

    nc = bacc.Bacc("TRN2")

    # Inputs are HOST-PREARRANGED into the exact SBUF tile layouts so each
    # weight is ONE dma_start with 4KB-contiguous descriptor rows and xT is
    # one start per column sweep -- per-dma_start descriptor generation on
    # the sync queue (~605ns each) was the startup gate with 34 starts.
    xT = nc.declare_dram_parameter("xT", [NQ, P, DC, SCW], bf16, isOutput=False)
    wqT = nc.declare_dram_parameter("wqT", [P, DC, f], bf16, isOutput=False)
    wkT = nc.declare_dram_parameter("wkT", [P, DC, f], bf16, isOutput=False)
    wvT = nc.declare_dram_parameter("wvT", [P, DC, f], bf16, isOutput=False)
    woT = nc.declare_dram_parameter("woT", [P, FC, d], bf16, isOutput=False)
    out = nc.declare_dram_parameter("out", [d, n], bf16, isOutput=True)
    # fc0 half of quarter 3's output projection -- written during quarter 3
    # and summed with out's fc1-only quarter-3 columns on the host, so the
    # tail never runs a PSUM-accumulate + add chain
    out2 = nc.declare_dram_parameter("out2", [d, 512], bf16, isOutput=True)

    with tile.TileContext(nc) as tc:
        with (
            tc.tile_pool(name="qkv", bufs=1) as qkv,
            tc.tile_pool(name="outT", bufs=1) as outp,
            tc.tile_pool(name="xw", bufs=1) as xw,
            tc.tile_pool(name="pt", bufs=3) as ptp,
            tc.tile_pool(name="ptb", bufs=3) as ptpb,
            tc.tile_pool(name="norm", bufs=2) as normp,
            tc.tile_pool(name="wosb", bufs=4) as wosbp,
            tc.tile_pool(name="scps", bufs=2, space="PSUM") as scps,
            tc.tile_pool(name="avps", bufs=2, space="PSUM") as avps,
            tc.tile_pool(name="gen", bufs=2, space="PSUM") as genp,
        ):
            QT_sb = qkv.tile([P, FC, n], bf16)
            # per-head K^T, zero-padded to a full 128-row stationary (head h
            # occupies partition rows po..po+hd, matching its rows in QT)
            KTz_sb = qkv.tile([P, hpc, n], bf16)
            V_sb = qkv.tile([P, NT, hpc, hd + 2], bf16)
            # one outT tile PER QUARTER: the tile framework tracks deps at
            # tile granularity, so a shared outT would serialize every wo
            # block on the most recent pass_end mul
            outTq = [
                outp.tile([P, FC, SCW], bf16, name=f"outTq{i}")
                for i in range(NQ)
            ]
            # sweep-major xT: slice [:, g, :, :] is contiguous per partition
            # (8KB rows) so one dma_start covers a whole column sweep
            xT_r = xw.tile([P, NQ, DC, SCW], bf16)
            wqT_r = xw.tile([P, DC, f], bf16)
            wkT_r = xw.tile([P, DC, f], bf16)
            wvT_r = xw.tile([P, DC, f], bf16)
            woT_sb = xw.tile([P, FC, d], bf16)

            # ones column of V_aug (accumulates softmax denominators in AV)
            nc.vector.memset(V_sb[:, :, :, hd : hd + 1], 1.0)
            nc.vector.memset(V_sb[:, :, :, hd + 1 : hd + 2], 0.0)

            def ktz_zero(g):
                """Zero the sweep-g columns of every head's padded K^T slab
                (just-in-time: only sweep 0 gates the first pass)."""
                nc.vector.memset(
                    KTz_sb[:, :, g * SCW : (g + 1) * SCW], 0.0
                )

            def xsweep(g):
                """xT column sweep g, split across the sync+gpsimd queues.
                Emitted JUST-IN-TIME before its first consumer: tile deps
                batch coarsely, so a consumer emitted after a dma_start
                waits for every earlier writer of that tile."""
                nc.sync.dma_start(
                    out=xT_r[:, g, 0 : DC // 2, :], in_=xT[g][:, 0 : DC // 2, :]
                )
                nc.gpsimd.dma_start(
                    out=xT_r[:, g, DC // 2 :, :], in_=xT[g][:, DC // 2 :, :]
                )

            # first-use-ordered input stream: weights serial on the scalar
            # queue (idle until the first exp ~9us in), sweeps on sync+gpsimd
            nc.scalar.dma_start(out=wqT_r[:], in_=wqT[:])
            xsweep(0)
            nc.sync.dma_start(out=wkT_r[:, 0 : DC // 2, :], in_=wkT[:, 0 : DC // 2, :])
            nc.gpsimd.dma_start(out=wkT_r[:, DC // 2 :, :], in_=wkT[:, DC // 2 :, :])
            nc.scalar.dma_start(out=wvT_r[:], in_=wvT[:])

            def qs(fc, qq):
                """Q^T projection stage: heads 2fc,2fc+1, q columns of
                quarter qq (dc-outer accumulation, one psum bank)."""
                ps = genp.tile([P, SCW], f32, tag="gen", name=f"q{fc}{qq}")
                for dc in range(DC):
                    nc.tensor.matmul(
                        ps[:],
                        wqT_r[:, dc, fc * P : (fc + 1) * P],
                        xT_r[:, qq, dc, :],
                        start=(dc == 0),
                        stop=(dc == DC - 1),
                    )
                nc.vector.tensor_copy(
                    QT_sb[:, fc, qq * SCW : (qq + 1) * SCW], ps[:]
                )

            def ks(fc, g):
                """K^T projection stage: heads 2fc,2fc+1, k columns of
                sweep g; rows land in each head's padded KTz slot."""
                ps = genp.tile([P, SCW], f32, tag="gen", name=f"k{fc}{g}")
                for dc in range(DC):
                    nc.tensor.matmul(
                        ps[:],
                        wkT_r[:, dc, fc * P : (fc + 1) * P],
                        xT_r[:, g, dc, :],
                        start=(dc == 0),
                        stop=(dc == DC - 1),
                    )
                sl = slice(g * SCW, (g + 1) * SCW)
                nc.vector.tensor_copy(
                    KTz_sb[0:hd, 2 * fc, sl], ps[0:hd, :]
                )
                nc.vector.tensor_copy(
                    KTz_sb[hd : 2 * hd, 2 * fc + 1, sl],
                    ps[hd : 2 * hd, :],
                )

            def v_tile(nt):
                """V tile nt in natural [n, feat] layout (stationary = xT
                chunk, moving = wv)."""
                ps = genp.tile([P, SCW], f32, tag="gen", name=f"v{nt}")
                g, j = nt // KG, nt % KG
                for dc in range(DC):
                    nc.tensor.matmul(
                        ps[:, 0:f],
                        xT_r[:, g, dc, j * P : (j + 1) * P],
                        wvT_r[:, dc, :],
                        start=(dc == 0),
                        stop=(dc == DC - 1),
                    )
                nc.vector.tensor_copy(
                    V_sb[:, nt, :, 0:hd],
                    ps[:, 0:f].rearrange("p (h e) -> p h e", h=hpc),
                )

            def pass_begin():
                # pav holds the deferred AV emitter: AV(kc) is emitted after
                # S(kc+1), one exp-period after its pt was written -- an AV
                # issued the moment its exp's semaphore fires reads pt while
                # the ACT's SBUF writes are still committing (+~130ns, seen
                # on nearly every AV of the projection-free quarters)
                return {
                    "avo": avps.tile([hd + 2, SCW], f32, tag="avo", name="avo"),
                    "pav": None,
                }

            def pass_blocks(pd, h, qq, kcs, pre_kc=None, q0=None, w=SCW):
                """scores^T -> exp -> AV accumulate for k-chunks `kcs`.
                q0/w override the q-column window (the final pass runs as
                two 256-wide sub-passes so its tail overlaps wo)."""
                fc = (h * hd) // P
                if q0 is None:
                    q0 = qq * SCW
                avo = pd["avo"]
                kcs = list(kcs)
                # kc PAIRS: one [128, 2, SCW] exp per pair (reads 2 psum
                # banks) halves the ACT instruction count -- ACT is the
                # pacer in the projection-free quarters
                for i in range(0, len(kcs), 2):
                    k0, k1 = kcs[i], kcs[i + 1]
                    sc = scps.tile([P, 2, SCW], f32, tag="sc")
                    for j, kc in ((0, k0), (1, k1)):
                        nc.tensor.matmul(
                            sc[:, j, 0:w],
                            KTz_sb[:, h, kc * P : (kc + 1) * P],
                            QT_sb[:, fc, q0 : q0 + w],
                            start=True,
                            stop=True,
                        )
                    # pre_kc (v_tiles) AFTER the scores pair: a v_tile
                    # waiting on the wv DMA must not block the exp stream
                    if pre_kc is not None:
                        pre_kc(k0)
                        pre_kc(k1)
                    if pd["pav"] is not None:
                        pd["pav"]()
                    pt = (ptp if k0 % 4 == 0 else ptpb).tile(
                        [P, 2, SCW], bf16, tag="pt"
                    )
                    nc.scalar.activation(
                        pt[:, :, 0:w], sc[:, :, 0:w],
                        mybir.ActivationFunctionType.Exp,
                        scale=scale,
                    )

                    def pav(k0=k0, k1=k1, pt=pt):
                        for j, kc in ((0, k0), (1, k1)):
                            nc.tensor.matmul(
                                avo[:, 0:w],
                                V_sb[:, kc, h, :],
                                pt[:, j, 0:w],
                                start=(kc == 0),
                                stop=(kc == NT - 1),
                            )

                    pd["pav"] = pav

            def pass_end(pd, h, qq, q0=None, w=SCW):
                if pd["pav"] is not None:
                    pd["pav"]()
                    pd["pav"] = None
                avo = pd["avo"]
                """Normalize rows 0..hd-1 of avo by row hd (softmax sums):
                approx-reciprocal the PSUM sums row (sums are O(1e3) --
                far from the approximation's edge cases), partition-
                broadcast, multiply straight out of PSUM into outT."""
                fc = (h * hd) // P
                po = (h * hd) % P
                if q0 is None:
                    q0 = qq * SCW
                o0 = q0 - qq * SCW
                sums = normp.tile([1, SCW], f32, tag="sums")
                nc.vector.tensor_copy(
                    sums[:, 0:w], avo[hd : hd + 1, o0 : o0 + w]
                )
                rrow = normp.tile([1, SCW], f32, tag="rrow")
                nc.vector.reciprocal_approx_fast(rrow[:, 0:w], sums[:, 0:w])
                bc = normp.tile([hd, SCW], f32, tag="bc")
                nc.gpsimd.partition_broadcast(bc[:, 0:w], rrow[:, 0:w])
                nc.vector.tensor_mul(
                    outTq[qq][po : po + hd, fc, o0 : o0 + w],
                    avo[0:hd, o0 : o0 + w],
                    bc[:, 0:w],
                )

            # Each pass's normalize chain is EMITTED near the END of the
            # next pass: chains emitted at their natural spot make every
            # later-emitted PE instruction coarse-wait on their mul. The
            # leftover AV flushes early (chunk 0) so its pt buffer frees,
            # while the chain lands before the last 2 kc so only those
            # scores sit behind the (long-since-computed) mul.
            pending_end = [None]

            def flush_av():
                if pending_end[0] is not None:
                    pd = pending_end[0][0]
                    if pd["pav"] is not None:
                        pd["pav"]()
                        pd["pav"] = None

            def flush_end():
                if pending_end[0] is not None:
                    pd, h, qq = pending_end[0]
                    pass_end(pd, h, qq)
                    pending_end[0] = None

            def do_pass(h, qq, pre_kc=None, mid=None):
                avo = pass_begin()
                pass_blocks(avo, h, qq, range(0, KG), pre_kc=pre_kc)
                flush_av()
                pass_blocks(avo, h, qq, range(KG, NT // 2), pre_kc=pre_kc)
                if mid is not None:
                    mid()
                pass_blocks(avo, h, qq, range(NT // 2, NT - 2), pre_kc=pre_kc)
                flush_end()
                pass_blocks(avo, h, qq, range(NT - 2, NT), pre_kc=pre_kc)
                pending_end[0] = (avo, h, qq)

            def wo_blocks(qq, dos, copy_eng="dve", pool=None):
                """Output projection for quarter qq, do-blocks `dos`
                (contract both fc chunks; emits the partial TRANSPOSED
                [d, n]). PSUM->SBUF copies alternate ACT/DVE so neither
                queue backs up ahead of the next quarter's exp/AV chain;
                copy_eng="act" keeps the DVE free (tail filler blocks run
                during the final normalize chain, which lives on DVE)."""
                q0 = qq * SCW
                for do in dos:
                    pl = pool if pool is not None else genp
                    tg = "sc" if pool is not None else "gen"
                    ps = pl.tile([P, SCW], f32, tag=tg, name=f"wo{do}")
                    for fc in range(FC):
                        nc.tensor.matmul(
                            ps[:],
                            woT_sb[:, fc, do * P : (do + 1) * P],
                            outTq[qq][:, fc, :],
                            start=(fc == 0),
                            stop=(fc == FC - 1),
                        )
                    ob = wosbp.tile([P, SCW], bf16, tag="ob")
                    if copy_eng == "act" or (copy_eng == "alt" and do % 2 == 0):
                        nc.scalar.activation(
                            ob[:], ps[:], mybir.ActivationFunctionType.Copy
                        )
                    else:
                        nc.vector.tensor_copy(ob[:], ps[:])
                    # sync-queue descriptors process ~4x faster than
                    # gpsimd-queue ones (42ns vs 155ns each, measured), and
                    # the input stream is done by the time wo runs
                    nc.sync.dma_start(
                        out=out[do * P : (do + 1) * P, q0 : q0 + SCW],
                        in_=ob[:],
                    )

            def wo_q3_fc0(dos):
                """The fc0 (heads 0,1) half of quarter 3's output
                projection, emitted inside pass(2,3) as soon as those heads
                are final. Lands in out2, summed on the host."""
                for do in dos:
                    ps = genp.tile([P, SCW], f32, tag="gen", name=f"w3a{do}")
                    nc.tensor.matmul(
                        ps[:],
                        woT_sb[:, 0, do * P : (do + 1) * P],
                        outTq[3][:, 0, :],
                        start=True,
                        stop=True,
                    )
                    ob = wosbp.tile([P, SCW], bf16, tag="ob")
                    nc.vector.tensor_copy(ob[:], ps[:])
                    nc.sync.dma_start(
                        out=out2[do * P : (do + 1) * P, :], in_=ob[:]
                    )

            # persistent tail output staging: fc1 halves land per-do, one
            # full-width DMA per do after its second half
            obq3 = outp.tile([P, d // P, SCW], bf16)

            def wo_q3_fc1(half, dos):
                """Tail: fc1-only wo for one 256-wide half of quarter 3
                (half A runs while half B's normalize chain is still in
                flight). Copies stay on ACT for half A (DVE owns the norm
                chains), alternate for half B."""
                q3 = 3 * SCW
                o0 = half * 256
                for do in dos:
                    ps = genp.tile([P, SCW], f32, tag="gen", name=f"w3b{do}")
                    nc.tensor.matmul(
                        ps[:, 0:256],
                        woT_sb[:, 1, do * P : (do + 1) * P],
                        outTq[3][:, 1, o0 : o0 + 256],
                        start=True,
                        stop=True,
                    )
                    if half == 0:
                        nc.scalar.activation(
                            obq3[:, do, o0 : o0 + 256], ps[:, 0:256],
                            mybir.ActivationFunctionType.Copy,
                        )
                    else:
                        nc.vector.tensor_copy(
                            obq3[:, do, o0 : o0 + 256], ps[:, 0:256]
                        )
                    if half == 1:
                        eng = nc.sync if do % 2 == 0 else nc.gpsimd
                        eng.dma_start(
                            out=out[do * P : (do + 1) * P, q3 : q3 + SCW],
                            in_=obq3[:, do, :],
                        )

            # ---- emission order = scheduling priority ----
            # quarter 0: the xT DMA stream is the gate; interleave the h0
            # and h1 passes sweep-by-sweep so every landed sweep unlocks
            # ~2x the PE work (both heads' scores + the fc1 projections
            # that only need sweep 0)
            ktz_zero(0)
            qs(0, 0)
            ks(0, 0)
            avo0 = pass_begin()
            pass_blocks(avo0, 0, 0, range(0, KG), pre_kc=v_tile)
            ks(1, 0)
            qs(1, 0)
            avo1 = pass_begin()
            pass_blocks(avo1, 1, 0, range(0, KG))
            xsweep(1)
            ktz_zero(1)
            ks(0, 1)
            pass_blocks(avo0, 0, 0, range(KG, 2 * KG), pre_kc=v_tile)
            ks(1, 1)
            pass_blocks(avo1, 1, 0, range(KG, 2 * KG))
            xsweep(2)
            ktz_zero(2)
            ks(0, 2)
            pass_blocks(avo0, 0, 0, range(2 * KG, 3 * KG), pre_kc=v_tile)
            ks(1, 2)
            pass_blocks(avo1, 1, 0, range(2 * KG, 3 * KG))
            xsweep(3)
            ktz_zero(3)
            ks(0, 3)
            pass_blocks(avo0, 0, 0, range(3 * KG, NT), pre_kc=v_tile)
            ks(1, 3)
            nc.sync.dma_start(out=woT_sb[:], in_=woT[:])
            pass_blocks(avo1, 1, 0, range(3 * KG, NT))
            pass_end(avo0, 0, 0)
            pending_end[0] = (avo1, 1, 0)

            do_pass(2, 0, mid=lambda: qs(0, 1))
            do_pass(3, 0, mid=lambda: qs(1, 1))
            # wo for a finished quarter is spread through the next quarter's
            # passes via the mid hook: cross-engine waits batch coarsely
            # (anything emitted after a pass_end waits on its mul), so the
            # blocks must be emitted BEFORE the surrounding pass_end
            def mids(*fns):
                return lambda: [fn() for fn in fns]

            do_pass(0, 1, mid=lambda: qs(0, 2))
            do_pass(1, 1, mid=lambda: qs(1, 2))
            do_pass(2, 1, mid=lambda: qs(0, 3))
            do_pass(3, 1, mid=lambda: qs(1, 3))
            for h in range(hpc):
                do_pass(h, 2, mid=lambda h=h: wo_blocks(0, [2 * h, 2 * h + 1]))
            do_pass(0, 3, mid=lambda: wo_blocks(1, [0, 1, 2, 3]))
            do_pass(1, 3, mid=lambda: wo_blocks(1, [4, 5, 6, 7]))
            # fc0's out2 stream rides pass(2,3): heads 0,1 are done, and
            # emitting it here lets its 1MB of output DMA drain during
            # compute instead of stacking onto the post-kernel drain
            do_pass(
                2, 3,
                mid=mids(
                    lambda: wo_blocks(2, [0, 1, 2, 3]),
                    lambda: wo_q3_fc0(range(8)),
                ),
            )
            # final pass. The tail is software-pipelined: the normalize is
            # split into two half-width chains, fc0 filler blocks keep the
            # PE warm through chain A, fc1's half-A wo overlaps chain B.
            q3 = 3 * SCW
            avoz = pass_begin()
            pass_blocks(avoz, 3, 3, range(0, KG))
            flush_av()
            pass_blocks(avoz, 3, 3, range(KG, NT // 2))
            wo_blocks(2, [4, 5], copy_eng="dve")
            pass_blocks(avoz, 3, 3, range(NT // 2, NT - 2))
            flush_end()
            pass_blocks(avoz, 3, 3, range(NT - 2, NT))
            wo_blocks(2, [6, 7], copy_eng="dve")
            pass_end(avoz, 3, 3, q0=q3, w=256)
            wo_q3_fc1(0, range(8))
            pass_end(avoz, 3, 3, q0=q3 + 256, w=256)
            wo_q3_fc1(1, range(8))
    nc.finalize()
    return nc


def make_in_maps(x, Wq, Wk, Wv, Wo):
    """Shard full inputs into per-core DRAM parameter maps (bf16)."""
    import ml_dtypes

    bf16 = ml_dtypes.bfloat16
    DC, NQ, SCW = D // P, N // 512, 512

    def w_pre(wT):  # [d_or_f, cols] -> [P, chunks, cols]
        return np.ascontiguousarray(
            wT.reshape(-1, P, wT.shape[1]).transpose(1, 0, 2)
        ).astype(bf16)

    x = np.asarray(x, dtype=np.float32)
    # [d, n] -> sweep-major [NQ, P, DC, SCW] matching the xT_r tile
    xTs = [
        np.ascontiguousarray(
            x[b].T.reshape(DC, P, NQ, SCW).transpose(2, 1, 0, 3)
        ).astype(bf16)
        for b in range(B)
    ]
    WqT = np.asarray(Wq, np.float32).T
    WkT = np.asarray(Wk, np.float32).T
    WvT = np.asarray(Wv, np.float32).T
    WoB = np.asarray(Wo, np.float32)
    in_maps = []
    for c in range(N_CORES):
        b, g = c // (N_CORES // B), c % (N_CORES // B)
        fs = slice(g * F, (g + 1) * F)
        in_maps.append(
            {
                "xT": xTs[b],
                "wqT": w_pre(WqT[:, fs]),
                "wkT": w_pre(WkT[:, fs]),
                "wvT": w_pre(WvT[:, fs]),
                "woT": w_pre(np.ascontiguousarray(WoB[:, fs].T)),
            }
        )
    return in_maps


_NC_CACHE = {}


def run(x, Wq, Wk, Wv, Wo, trace=False):
    from concourse.bass_utils import run_bass_kernel_spmd

    # NOTE: walrus --enable-ldw-opt stays at its default (false): v2 has no
    # consecutive matmuls sharing a stationary (SCW == QB == 512), and the
    # bf16 Ldweights form is rejected by the opt's codegen path anyway.
    if "nc" not in _NC_CACHE:
        _NC_CACHE["nc"] = build_nc()
    nc = _NC_CACHE["nc"]
    in_maps = make_in_maps(x, Wq, Wk, Wv, Wo)
    res = run_bass_kernel_spmd(nc, in_maps, core_ids=list(range(N_CORES)), trace=trace)
    parts = []
    for i in range(N_CORES):
        p = np.asarray(res.results[i]["out"]).astype(np.float32)
        # quarter 3's output was emitted in two halves: fc1 went to out,
        # fc0 to out2 -- recombine here
        p[:, 3 * 512 :] += np.asarray(res.results[i]["out2"]).astype(
            np.float32
        )
        parts.append(p)
    gpb = N_CORES // B
    # per-core partials are transposed [d, n]: sum the group, then untranspose
    full = np.stack(
        [
            sum(parts[b * gpb + 1 : (b + 1) * gpb], parts[b * gpb]).T
            for b in range(B)
        ]
    )
    return np.ascontiguousarray(full, dtype=np.float32), res


def kernel(x, Wq, bq, Wk, bk, Wv, bv, Wo, bo):
    full, _ = run(x, Wq, Wk, Wv, Wo)
    return full


# revision 67
# speedup vs baseline: 1.0240x; 1.0022x over previous
"""Multi-head attention kernel for 8 TRN2 NeuronCores.

Problem: b=2, n=2048, d=1024, heads=16, hd=64.
  q/k/v = x @ W{q,k,v}.T (+ zero bias)
  per head: softmax(q k^T / sqrt(d)) @ v
  out = concat @ Wo.T (+ zero bias)

Sharding (8 cores): data-parallel over batch (2) x tensor-parallel over
heads (16 heads -> 4 groups of 4). Core c handles batch c//4, heads
4*(c%4) .. 4*(c%4)+3 (feature slice of 256 columns). Wo is applied
row-parallel: each core emits a partial output; the host sums the 4
partials per batch (and untransposes). No collectives needed.

v2 structure (measured-v1 post-mortem):
 - everything bf16 on SBUF/DRAM (PSUM accumulation stays f32): halves
   input DMA (startup was DMA-gated ~12us), halves SBUF, keeps the PE
   at the same 1 cyc/row as f32r but without the moving>=256 caveat.
 - attention runs in q-QUARTERS (SCW=512): passes are (quarter, head)
   ordered, so the output projection for quarter qq runs as soon as the
   4 heads of qq are done -- the old version ran ALL of wo_half(1) after
   the last pass, leaving a ~15us PE-only tail at degraded clock.
 - scores psum pool has 3 bufs + pt pool 3 bufs so the scheduler runs
   scores ~2 kc ahead of the ACT exp stream (absorbs exp jitter).
 - normalize chain per pass_end is row-copy(DVE) -> partition_broadcast
   of the raw sums (GpSimd) -> reciprocal on 64 partitions (DVE) ->
   multiply reading avo straight from PSUM. The v1 chain bounced the
   sums through two SBUF DMAs (reciprocal-then-broadcast) which cost
   ~9us of serial latency at the tail.
 - K^T is stored zero-padded per head to a full 128-row stationary
   (K=64 matmuls run at 2 cyc/row on HW; padded K=128 runs at 1).
 - V is built in natural [n, feat] layout with a ones column appended
   (the ones column accumulates softmax denominators during AV).
 - projections stream behind a column-split xT DMA (quarter 0 columns
   of every contraction chunk land first so the first Q/K stages and
   the first pass start ~3us in).

Biases are structurally zero in this problem spec and are skipped.
"""

import numpy as np

HEADS = 16
D = 1024
N = 2048
B = 2
N_CORES = 8
HPC = HEADS // (N_CORES // B)  # heads per core = 4
HD = D // HEADS                # 64
F = HPC * HD                   # 256 features per core
P = 128


def build_nc(n=N, d=D, hpc=HPC, hd=HD):
    """Build the per-core Bass program (SPMD: same program on all 8 cores)."""
    import concourse.bass as bass
    import concourse.tile as tile
    from concourse import bacc, mybir

    f32 = mybir.dt.float32
    bf16 = mybir.dt.bfloat16
    f = hpc * hd            # per-core feature count (256)
    FC = f // P             # feature chunks (2)
    DC = d // P             # contraction chunks over d (8)
    NT = n // P             # n tiles / k chunks (16)
    SCW = 512               # scores width = one q-quarter = one psum bank
    NQ = n // SCW           # q-quarters (4)
    KG = NT // NQ           # k-chunks per xT column sweep (4)
    scale = 1.0 / float(np.sqrt(np.float32(d)))

    nc = bacc.Bacc("TRN2")

    # Inputs are HOST-PREARRANGED into the exact SBUF tile layouts so each
    # weight is ONE dma_start with 4KB-contiguous descriptor rows and xT is
    # one start per column sweep -- per-dma_start descriptor generation on
    # the sync queue (~605ns each) was the startup gate with 34 starts.
    xT = nc.declare_dram_parameter("xT", [NQ, P, DC, SCW], bf16, isOutput=False)
    wqT = nc.declare_dram_parameter("wqT", [P, DC, f], bf16, isOutput=False)
    wkT = nc.declare_dram_parameter("wkT", [P, DC, f], bf16, isOutput=False)
    wvT = nc.declare_dram_parameter("wvT", [P, DC, f], bf16, isOutput=False)
    woT = nc.declare_dram_parameter("woT", [P, FC, d], bf16, isOutput=False)
    out = nc.declare_dram_parameter("out", [d, n], bf16, isOutput=True)
    # fc0 half of quarter 3's output projection -- written during quarter 3
    # and summed with out's fc1-only quarter-3 columns on the host, so the
    # tail never runs a PSUM-accumulate + add chain
    out2 = nc.declare_dram_parameter("out2", [d, 512], bf16, isOutput=True)

    with tile.TileContext(nc) as tc:
        with (
            tc.tile_pool(name="qkv", bufs=1) as qkv,
            tc.tile_pool(name="outT", bufs=1) as outp,
            tc.tile_pool(name="xw", bufs=1) as xw,
            tc.tile_pool(name="pt", bufs=3) as ptp,
            tc.tile_pool(name="ptb", bufs=3) as ptpb,
            tc.tile_pool(name="norm", bufs=2) as normp,
            tc.tile_pool(name="wosb", bufs=4) as wosbp,
            tc.tile_pool(name="scps", bufs=2, space="PSUM") as scps,
            tc.tile_pool(name="avps", bufs=2, space="PSUM") as avps,
            tc.tile_pool(name="gen", bufs=2, space="PSUM") as genp,
        ):
            QT_sb = qkv.tile([P, FC, n], bf16)
            # per-head K^T, zero-padded to a full 128-row stationary (head h
            # occupies partition rows po..po+hd, matching its rows in QT)
            KTz_sb = qkv.tile([P, hpc, n], bf16)
            V_sb = qkv.tile([P, NT, hpc, hd + 2], bf16)
            # one outT tile PER QUARTER: the tile framework tracks deps at
            # tile granularity, so a shared outT would serialize every wo
            # block on the most recent pass_end mul
            outTq = [
                outp.tile([P, FC, SCW], bf16, name=f"outTq{i}")
                for i in range(NQ)
            ]
            # sweep-major xT: slice [:, g, :, :] is contiguous per partition
            # (8KB rows) so one dma_start covers a whole column sweep
            xT_r = xw.tile([P, NQ, DC, SCW], bf16)
            wqT_r = xw.tile([P, DC, f], bf16)
            wkT_r = xw.tile([P, DC, f], bf16)
            wvT_r = xw.tile([P, DC, f], bf16)
            woT_sb = xw.tile([P, FC, d], bf16)

            # ones column of V_aug (accumulates softmax denominators in AV)
            nc.vector.memset(V_sb[:, :, :, hd : hd + 1], 1.0)
            nc.vector.memset(V_sb[:, :, :, hd + 1 : hd + 2], 0.0)

            def ktz_zero(g):
                """Zero the sweep-g columns of every head's padded K^T slab
                (just-in-time: only sweep 0 gates the first pass)."""
                nc.vector.memset(
                    KTz_sb[:, :, g * SCW : (g + 1) * SCW], 0.0
                )

            def xsweep(g):
                """xT column sweep g, split across the sync+gpsimd queues.
                Emitted JUST-IN-TIME before its first consumer: tile deps
                batch coarsely, so a consumer emitted after a dma_start
                waits for every earlier writer of that tile."""
                nc.sync.dma_start(
                    out=xT_r[:, g, 0 : DC // 2, :], in_=xT[g][:, 0 : DC // 2, :]
                )
                nc.gpsimd.dma_start(
                    out=xT_r[:, g, DC // 2 :, :], in_=xT[g][:, DC // 2 :, :]
                )

            # first-use-ordered input stream: weights serial on the scalar
            # queue (idle until the first exp ~9us in), sweeps on sync+gpsimd
            nc.scalar.dma_start(out=wqT_r[:], in_=wqT[:])
            xsweep(0)
            nc.sync.dma_start(out=wkT_r[:, 0 : DC // 2, :], in_=wkT[:, 0 : DC // 2, :])
            nc.gpsimd.dma_start(out=wkT_r[:, DC // 2 :, :], in_=wkT[:, DC // 2 :, :])
            nc.scalar.dma_start(out=wvT_r[:], in_=wvT[:])

            def qs(fc, qq):
                """Q^T projection stage: heads 2fc,2fc+1, q columns of
                quarter qq (dc-outer accumulation, one psum bank)."""
                ps = genp.tile([P, SCW], f32, tag="gen", name=f"q{fc}{qq}")
                for dc in range(DC):
                    nc.tensor.matmul(
                        ps[:],
                        wqT_r[:, dc, fc * P : (fc + 1) * P],
                        xT_r[:, qq, dc, :],
                        start=(dc == 0),
                        stop=(dc == DC - 1),
                    )
                nc.vector.tensor_copy(
                    QT_sb[:, fc, qq * SCW : (qq + 1) * SCW], ps[:]
                )

            def ks(fc, g):
                """K^T projection stage: heads 2fc,2fc+1, k columns of
                sweep g; rows land in each head's padded KTz slot."""
                ps = genp.tile([P, SCW], f32, tag="gen", name=f"k{fc}{g}")
                for dc in range(DC):
                    nc.tensor.matmul(
                        ps[:],
                        wkT_r[:, dc, fc * P : (fc + 1) * P],
                        xT_r[:, g, dc, :],
                        start=(dc == 0),
                        stop=(dc == DC - 1),
                    )
                sl = slice(g * SCW, (g + 1) * SCW)
                nc.vector.tensor_copy(
                    KTz_sb[0:hd, 2 * fc, sl], ps[0:hd, :]
                )
                nc.vector.tensor_copy(
                    KTz_sb[hd : 2 * hd, 2 * fc + 1, sl],
                    ps[hd : 2 * hd, :],
                )

            def v_tile(nt):
                """V tile nt in natural [n, feat] layout (stationary = xT
                chunk, moving = wv)."""
                ps = genp.tile([P, SCW], f32, tag="gen", name=f"v{nt}")
                g, j = nt // KG, nt % KG
                for dc in range(DC):
                    nc.tensor.matmul(
                        ps[:, 0:f],
                        xT_r[:, g, dc, j * P : (j + 1) * P],
                        wvT_r[:, dc, :],
                        start=(dc == 0),
                        stop=(dc == DC - 1),
                    )
                nc.vector.tensor_copy(
                    V_sb[:, nt, :, 0:hd],
                    ps[:, 0:f].rearrange("p (h e) -> p h e", h=hpc),
                )

            def pass_begin():
                # pav holds the deferred AV emitter: AV(kc) is emitted after
                # S(kc+1), one exp-period after its pt was written -- an AV
                # issued the moment its exp's semaphore fires reads pt while
                # the ACT's SBUF writes are still committing (+~130ns, seen
                # on nearly every AV of the projection-free quarters)
                return {
                    "avo": avps.tile([hd + 2, SCW], f32, tag="avo", name="avo"),
                    "pav": None,
                }

            def pass_blocks(pd, h, qq, kcs, pre_kc=None, q0=None, w=SCW):
                """scores^T -> exp -> AV accumulate for k-chunks `kcs`.
                q0/w override the q-column window (the final pass runs as
                two 256-wide sub-passes so its tail overlaps wo)."""
                fc = (h * hd) // P
                if q0 is None:
                    q0 = qq * SCW
                avo = pd["avo"]
                kcs = list(kcs)
                # kc PAIRS: one [128, 2, SCW] exp per pair (reads 2 psum
                # banks) halves the ACT instruction count -- ACT is the
                # pacer in the projection-free quarters
                for i in range(0, len(kcs), 2):
                    k0, k1 = kcs[i], kcs[i + 1]
                    sc = scps.tile([P, 2, SCW], f32, tag="sc")
                    for j, kc in ((0, k0), (1, k1)):
                        nc.tensor.matmul(
                            sc[:, j, 0:w],
                            KTz_sb[:, h, kc * P : (kc + 1) * P],
                            QT_sb[:, fc, q0 : q0 + w],
                            start=True,
                            stop=True,
                        )
                    # pre_kc (v_tiles) AFTER the scores pair: a v_tile
                    # waiting on the wv DMA must not block the exp stream
                    if pre_kc is not None:
                        pre_kc(k0)
                        pre_kc(k1)
                    if pd["pav"] is not None:
                        pd["pav"]()
                    pt = (ptp if k0 % 4 == 0 else ptpb).tile(
                        [P, 2, SCW], bf16, tag="pt"
                    )
                    nc.scalar.activation(
                        pt[:, :, 0:w], sc[:, :, 0:w],
                        mybir.ActivationFunctionType.Exp,
                        scale=scale,
                    )

                    def pav(k0=k0, k1=k1, pt=pt):
                        for j, kc in ((0, k0), (1, k1)):
                            nc.tensor.matmul(
                                avo[:, 0:w],
                                V_sb[:, kc, h, :],
                                pt[:, j, 0:w],
                                start=(kc == 0),
                                stop=(kc == NT - 1),
                            )

                    pd["pav"] = pav

            def pass_end(pd, h, qq, q0=None, w=SCW):
                if pd["pav"] is not None:
                    pd["pav"]()
                    pd["pav"] = None
                avo = pd["avo"]
                """Normalize rows 0..hd-1 of avo by row hd (softmax sums):
                approx-reciprocal the PSUM sums row (sums are O(1e3) --
                far from the approximation's edge cases), partition-
                broadcast, multiply straight out of PSUM into outT."""
                fc = (h * hd) // P
                po = (h * hd) % P
                if q0 is None:
                    q0 = qq * SCW
                o0 = q0 - qq * SCW
                sums = normp.tile([1, SCW], f32, tag="sums")
                nc.vector.tensor_copy(
                    sums[:, 0:w], avo[hd : hd + 1, o0 : o0 + w]
                )
                rrow = normp.tile([1, SCW], f32, tag="rrow")
                nc.vector.reciprocal_approx_fast(rrow[:, 0:w], sums[:, 0:w])
                bc = normp.tile([hd, SCW], f32, tag="bc")
                nc.gpsimd.partition_broadcast(bc[:, 0:w], rrow[:, 0:w])
                nc.vector.tensor_mul(
                    outTq[qq][po : po + hd, fc, o0 : o0 + w],
                    avo[0:hd, o0 : o0 + w],
                    bc[:, 0:w],
                )

            # Each pass's normalize chain is EMITTED near the END of the
            # next pass: chains emitted at their natural spot make every
            # later-emitted PE instruction coarse-wait on their mul. The
            # leftover AV flushes early (chunk 0) so its pt buffer frees,
            # while the chain lands before the last 2 kc so only those
            # scores sit behind the (long-since-computed) mul.
            pending_end = [None]

            def flush_av():
                if pending_end[0] is not None:
                    pd = pending_end[0][0]
                    if pd["pav"] is not None:
                        pd["pav"]()
                        pd["pav"] = None

            def flush_end():
                if pending_end[0] is not None:
                    pd, h, qq = pending_end[0]
                    pass_end(pd, h, qq)
                    pending_end[0] = None

            def do_pass(h, qq, pre_kc=None, mid=None):
                avo = pass_begin()
                pass_blocks(avo, h, qq, range(0, KG), pre_kc=pre_kc)
                flush_av()
                pass_blocks(avo, h, qq, range(KG, NT // 2), pre_kc=pre_kc)
                if mid is not None:
                    mid()
                pass_blocks(avo, h, qq, range(NT // 2, NT - 2), pre_kc=pre_kc)
                flush_end()
                pass_blocks(avo, h, qq, range(NT - 2, NT), pre_kc=pre_kc)
                pending_end[0] = (avo, h, qq)

            def wo_blocks(qq, dos, copy_eng="dve", pool=None):
                """Output projection for quarter qq, do-blocks `dos`
                (contract both fc chunks; emits the partial TRANSPOSED
                [d, n]). PSUM->SBUF copies alternate ACT/DVE so neither
                queue backs up ahead of the next quarter's exp/AV chain;
                copy_eng="act" keeps the DVE free (tail filler blocks run
                during the final normalize chain, which lives on DVE)."""
                q0 = qq * SCW
                for do in dos:
                    pl = pool if pool is not None else genp
                    tg = "sc" if pool is not None else "gen"
                    ps = pl.tile([P, SCW], f32, tag=tg, name=f"wo{do}")
                    for fc in range(FC):
                        nc.tensor.matmul(
                            ps[:],
                            woT_sb[:, fc, do * P : (do + 1) * P],
                            outTq[qq][:, fc, :],
                            start=(fc == 0),
                            stop=(fc == FC - 1),
                        )
                    ob = wosbp.tile([P, SCW], bf16, tag="ob")
                    if copy_eng == "act" or (copy_eng == "alt" and do % 2 == 0):
                        nc.scalar.activation(
                            ob[:], ps[:], mybir.ActivationFunctionType.Copy
                        )
                    else:
                        nc.vector.tensor_copy(ob[:], ps[:])
                    # sync-queue descriptors process ~4x faster than
                    # gpsimd-queue ones (42ns vs 155ns each, measured), and
                    # the input stream is done by the time wo runs
                    nc.sync.dma_start(
                        out=out[do * P : (do + 1) * P, q0 : q0 + SCW],
                        in_=ob[:],
                    )

            def wo_q3_fc0(dos):
                """The fc0 (heads 0,1) half of quarter 3's output
                projection, emitted inside pass(2,3) as soon as those heads
                are final. Lands in out2, summed on the host."""
                for do in dos:
                    ps = genp.tile([P, SCW], f32, tag="gen", name=f"w3a{do}")
                    nc.tensor.matmul(
                        ps[:],
                        woT_sb[:, 0, do * P : (do + 1) * P],
                        outTq[3][:, 0, :],
                        start=True,
                        stop=True,
                    )
                    ob = wosbp.tile([P, SCW], bf16, tag="ob")
                    nc.vector.tensor_copy(ob[:], ps[:])
                    nc.sync.dma_start(
                        out=out2[do * P : (do + 1) * P, :], in_=ob[:]
                    )

            # persistent tail output staging: fc1 halves land per-do, one
            # full-width DMA per do after its second half
            obq3 = outp.tile([P, d // P, SCW], bf16)

            def wo_q3_fc1(half, dos):
                """Tail: fc1-only wo for one 256-wide half of quarter 3
                (half A runs while half B's normalize chain is still in
                flight). Copies stay on ACT for half A (DVE owns the norm
                chains), alternate for half B."""
                q3 = 3 * SCW
                o0 = half * 256
                for do in dos:
                    ps = genp.tile([P, SCW], f32, tag="gen", name=f"w3b{do}")
                    nc.tensor.matmul(
                        ps[:, 0:256],
                        woT_sb[:, 1, do * P : (do + 1) * P],
                        outTq[3][:, 1, o0 : o0 + 256],
                        start=True,
                        stop=True,
                    )
                    if do % 2 == 0:
                        nc.scalar.activation(
                            obq3[:, do, o0 : o0 + 256], ps[:, 0:256],
                            mybir.ActivationFunctionType.Copy,
                        )
                    else:
                        nc.vector.tensor_copy(
                            obq3[:, do, o0 : o0 + 256], ps[:, 0:256]
                        )
                    if half == 1:
                        eng = nc.sync if do % 2 == 0 else nc.gpsimd
                        eng.dma_start(
                            out=out[do * P : (do + 1) * P, q3 : q3 + SCW],
                            in_=obq3[:, do, :],
                        )

            # ---- emission order = scheduling priority ----
            # quarter 0: the xT DMA stream is the gate; interleave the h0
            # and h1 passes sweep-by-sweep so every landed sweep unlocks
            # ~2x the PE work (both heads' scores + the fc1 projections
            # that only need sweep 0)
            ktz_zero(0)
            qs(0, 0)
            ks(0, 0)
            avo0 = pass_begin()
            pass_blocks(avo0, 0, 0, range(0, KG), pre_kc=v_tile)
            ks(1, 0)
            qs(1, 0)
            avo1 = pass_begin()
            pass_blocks(avo1, 1, 0, range(0, KG))
            xsweep(1)
            ktz_zero(1)
            ks(0, 1)
            pass_blocks(avo0, 0, 0, range(KG, 2 * KG), pre_kc=v_tile)
            ks(1, 1)
            pass_blocks(avo1, 1, 0, range(KG, 2 * KG))
            xsweep(2)
            ktz_zero(2)
            ks(0, 2)
            pass_blocks(avo0, 0, 0, range(2 * KG, 3 * KG), pre_kc=v_tile)
            ks(1, 2)
            pass_blocks(avo1, 1, 0, range(2 * KG, 3 * KG))
            xsweep(3)
            ktz_zero(3)
            ks(0, 3)
            pass_blocks(avo0, 0, 0, range(3 * KG, NT), pre_kc=v_tile)
            ks(1, 3)
            nc.sync.dma_start(out=woT_sb[:], in_=woT[:])
            pass_blocks(avo1, 1, 0, range(3 * KG, NT))
            pass_end(avo0, 0, 0)
            pending_end[0] = (avo1, 1, 0)

            do_pass(2, 0, mid=lambda: qs(0, 1))
            do_pass(3, 0, mid=lambda: qs(1, 1))
            # wo for a finished quarter is spread through the next quarter's
            # passes via the mid hook: cross-engine waits batch coarsely
            # (anything emitted after a pass_end waits on its mul), so the
            # blocks must be emitted BEFORE the surrounding pass_end
            def mids(*fns):
                return lambda: [fn() for fn in fns]

            do_pass(0, 1, mid=lambda: qs(0, 2))
            do_pass(1, 1, mid=lambda: qs(1, 2))
            do_pass(2, 1, mid=lambda: qs(0, 3))
            do_pass(3, 1, mid=lambda: qs(1, 3))
            for h in range(hpc):
                do_pass(h, 2, mid=lambda h=h: wo_blocks(0, [2 * h, 2 * h + 1]))
            do_pass(0, 3, mid=lambda: wo_blocks(1, [0, 1, 2, 3]))
            do_pass(1, 3, mid=lambda: wo_blocks(1, [4, 5, 6, 7]))
            # fc0's out2 stream rides pass(2,3): heads 0,1 are done, and
            # emitting it here lets its 1MB of output DMA drain during
            # compute instead of stacking onto the post-kernel drain
            do_pass(
                2, 3,
                mid=mids(
                    lambda: wo_blocks(2, [0, 1, 2, 3]),
                    lambda: wo_q3_fc0(range(8)),
                ),
            )
            # final pass. The tail is software-pipelined: the normalize is
            # split into two half-width chains, fc0 filler blocks keep the
            # PE warm through chain A, fc1's half-A wo overlaps chain B.
            q3 = 3 * SCW
            avoz = pass_begin()
            pass_blocks(avoz, 3, 3, range(0, KG))
            flush_av()
            pass_blocks(avoz, 3, 3, range(KG, NT // 2))
            wo_blocks(2, [4, 5], copy_eng="dve")
            pass_blocks(avoz, 3, 3, range(NT // 2, NT - 2))
            flush_end()
            pass_blocks(avoz, 3, 3, range(NT - 2, NT))
            wo_blocks(2, [6, 7], copy_eng="dve")
            pass_end(avoz, 3, 3, q0=q3, w=256)
            wo_q3_fc1(0, range(8))
            pass_end(avoz, 3, 3, q0=q3 + 256, w=256)
            wo_q3_fc1(1, range(8))
    nc.finalize()
    return nc


def make_in_maps(x, Wq, Wk, Wv, Wo):
    """Shard full inputs into per-core DRAM parameter maps (bf16)."""
    import ml_dtypes

    bf16 = ml_dtypes.bfloat16
    DC, NQ, SCW = D // P, N // 512, 512

    def w_pre(wT):  # [d_or_f, cols] -> [P, chunks, cols]
        return np.ascontiguousarray(
            wT.reshape(-1, P, wT.shape[1]).transpose(1, 0, 2)
        ).astype(bf16)

    x = np.asarray(x, dtype=np.float32)
    # [d, n] -> sweep-major [NQ, P, DC, SCW] matching the xT_r tile
    xTs = [
        np.ascontiguousarray(
            x[b].T.reshape(DC, P, NQ, SCW).transpose(2, 1, 0, 3)
        ).astype(bf16)
        for b in range(B)
    ]
    WqT = np.asarray(Wq, np.float32).T
    WkT = np.asarray(Wk, np.float32).T
    WvT = np.asarray(Wv, np.float32).T
    WoB = np.asarray(Wo, np.float32)
    in_maps = []
    for c in range(N_CORES):
        b, g = c // (N_CORES // B), c % (N_CORES // B)
        fs = slice(g * F, (g + 1) * F)
        in_maps.append(
            {
                "xT": xTs[b],
                "wqT": w_pre(WqT[:, fs]),
                "wkT": w_pre(WkT[:, fs]),
                "wvT": w_pre(WvT[:, fs]),
                "woT": w_pre(np.ascontiguousarray(WoB[:, fs].T)),
            }
        )
    return in_maps


_NC_CACHE = {}


def run(x, Wq, Wk, Wv, Wo, trace=False):
    from concourse.bass_utils import run_bass_kernel_spmd

    # NOTE: walrus --enable-ldw-opt stays at its default (false): v2 has no
    # consecutive matmuls sharing a stationary (SCW == QB == 512), and the
    # bf16 Ldweights form is rejected by the opt's codegen path anyway.
    if "nc" not in _NC_CACHE:
        _NC_CACHE["nc"] = build_nc()
    nc = _NC_CACHE["nc"]
    in_maps = make_in_maps(x, Wq, Wk, Wv, Wo)
    res = run_bass_kernel_spmd(nc, in_maps, core_ids=list(range(N_CORES)), trace=trace)
    parts = []
    for i in range(N_CORES):
        p = np.asarray(res.results[i]["out"]).astype(np.float32)
        # quarter 3's output was emitted in two halves: fc1 went to out,
        # fc0 to out2 -- recombine here
        p[:, 3 * 512 :] += np.asarray(res.results[i]["out2"]).astype(
            np.float32
        )
        parts.append(p)
    gpb = N_CORES // B
    # per-core partials are transposed [d, n]: sum the group, then untranspose
    full = np.stack(
        [
            sum(parts[b * gpb + 1 : (b + 1) * gpb], parts[b * gpb]).T
            for b in range(B)
        ]
    )
    return np.ascontiguousarray(full, dtype=np.float32), res


def kernel(x, Wq, bq, Wk, bk, Wv, bv, Wo, bo):
    full, _ = run(x, Wq, Wk, Wv, Wo)
    return full


# revision 68
# speedup vs baseline: 1.0249x; 1.0008x over previous
"""Multi-head attention kernel for 8 TRN2 NeuronCores.

Problem: b=2, n=2048, d=1024, heads=16, hd=64.
  q/k/v = x @ W{q,k,v}.T (+ zero bias)
  per head: softmax(q k^T / sqrt(d)) @ v
  out = concat @ Wo.T (+ zero bias)

Sharding (8 cores): data-parallel over batch (2) x tensor-parallel over
heads (16 heads -> 4 groups of 4). Core c handles batch c//4, heads
4*(c%4) .. 4*(c%4)+3 (feature slice of 256 columns). Wo is applied
row-parallel: each core emits a partial output; the host sums the 4
partials per batch (and untransposes). No collectives needed.

Structure (all decisions trace-measured on HW; 267.6us -> 210.6us):
 - everything bf16 on SBUF/DRAM (PSUM accumulation stays f32): same
   1 cyc/row PE rate as f32r, half the DMA/SBUF. rel err ~4.4e-3.
 - attention runs in q-QUARTERS (SCW=512), (quarter, head) ordered, so
   each quarter's output projection streams out during the NEXT quarter
   instead of lumping PE work and output DMA at the kernel tail.
 - the ACT engine is the pacer of projection-free stretches (an exp
   pair [128,2,512] costs ~1.1us vs the pair's 0.86us of PE work), so:
   exps are kc-PAIRED (halves ACT instruction count), all PSUM->SBUF
   output copies live on DVE/ACT chosen by phase, and the wo blocks are
   spread into the ACT-bound quarters as PE filler.
 - inputs are host-prearranged to the exact SBUF tile layouts: one
   dma_start per weight / per xT column sweep (4KB+ descriptor rows),
   split across the sync/gpsimd/scalar queues. Cross-engine deps batch
   coarsely (a consumer emitted after a dma_start waits on ALL earlier
   writers of that tile), so every dma_start is emitted just-in-time
   before its first consumer, and quarter 0 interleaves the h0/h1
   passes sweep-by-sweep to track the ~130GB/s input stream.
 - per-pass softmax normalize: row-copy sums (DVE) -> approx-reciprocal
   (custom DVE op, input must be SBUF) -> partition_broadcast (GpSimd)
   -> multiply straight out of avo PSUM. Chains are emitted near the
   END of the next pass (coarse dep batching again: emitted earlier,
   every later PE instruction waits on the mul); the leftover deferred
   AV flushes at the next pass's start to free its pt buffer.
 - K^T is zero-padded per head to a full 128-row stationary (K=64
   matmuls run 2 cyc/row on HW; padded K=128 runs 1). V is natural
   [n, feat] with a ones column (accumulates softmax denominators
   during AV, one PSUM row below the AV outputs).
 - the tail is software-pipelined: the final pass's normalize runs as
   two 256-wide chains; quarter 3's fc0 wo half lands in out2 during
   pass(2,3) (host sums it), fc1 halves interleave with the chains,
   out DMA descriptor gen alternates sync/gpsimd queues.

Biases are structurally zero in this problem spec and are skipped.
"""

import numpy as np

HEADS = 16
D = 1024
N = 2048
B = 2
N_CORES = 8
HPC = HEADS // (N_CORES // B)  # heads per core = 4
HD = D // HEADS                # 64
F = HPC * HD                   # 256 features per core
P = 128


def build_nc(n=N, d=D, hpc=HPC, hd=HD):
    """Build the per-core Bass program (SPMD: same program on all 8 cores)."""
    import concourse.bass as bass
    import concourse.tile as tile
    from concourse import bacc, mybir

    f32 = mybir.dt.float32
    bf16 = mybir.dt.bfloat16
    f = hpc * hd            # per-core feature count (256)
    FC = f // P             # feature chunks (2)
    DC = d // P             # contraction chunks over d (8)
    NT = n // P             # n tiles / k chunks (16)
    SCW = 512               # scores width = one q-quarter = one psum bank
    NQ = n // SCW           # q-quarters (4)
    KG = NT // NQ           # k-chunks per xT column sweep (4)
    scale = 1.0 / float(np.sqrt(np.float32(d)))

    nc = bacc.Bacc("TRN2")

    # Inputs are HOST-PREARRANGED into the exact SBUF tile layouts so each
    # weight is ONE dma_start with 4KB-contiguous descriptor rows and xT is
    # one start per column sweep -- per-dma_start descriptor generation on
    # the sync queue (~605ns each) was the startup gate with 34 starts.
    xT = nc.declare_dram_parameter("xT", [NQ, P, DC, SCW], bf16, isOutput=False)
    wqT = nc.declare_dram_parameter("wqT", [P, DC, f], bf16, isOutput=False)
    wkT = nc.declare_dram_parameter("wkT", [P, DC, f], bf16, isOutput=False)
    wvT = nc.declare_dram_parameter("wvT", [P, DC, f], bf16, isOutput=False)
    woT = nc.declare_dram_parameter("woT", [P, FC, d], bf16, isOutput=False)
    out = nc.declare_dram_parameter("out", [d, n], bf16, isOutput=True)
    # fc0 half of quarter 3's output projection -- written during quarter 3
    # and summed with out's fc1-only quarter-3 columns on the host, so the
    # tail never runs a PSUM-accumulate + add chain
    out2 = nc.declare_dram_parameter("out2", [d, 512], bf16, isOutput=True)

    with tile.TileContext(nc) as tc:
        with (
            tc.tile_pool(name="qkv", bufs=1) as qkv,
            tc.tile_pool(name="outT", bufs=1) as outp,
            tc.tile_pool(name="xw", bufs=1) as xw,
            tc.tile_pool(name="pt", bufs=3) as ptp,
            tc.tile_pool(name="ptb", bufs=3) as ptpb,
            tc.tile_pool(name="norm", bufs=2) as normp,
            tc.tile_pool(name="wosb", bufs=4) as wosbp,
            tc.tile_pool(name="scps", bufs=2, space="PSUM") as scps,
            tc.tile_pool(name="avps", bufs=2, space="PSUM") as avps,
            tc.tile_pool(name="gen", bufs=2, space="PSUM") as genp,
        ):
            QT_sb = qkv.tile([P, FC, n], bf16)
            # per-head K^T, zero-padded to a full 128-row stationary (head h
            # occupies partition rows po..po+hd, matching its rows in QT)
            KTz_sb = qkv.tile([P, hpc, n], bf16)
            V_sb = qkv.tile([P, NT, hpc, hd + 2], bf16)
            # one outT tile PER QUARTER: the tile framework tracks deps at
            # tile granularity, so a shared outT would serialize every wo
            # block on the most recent pass_end mul
            outTq = [
                outp.tile([P, FC, SCW], bf16, name=f"outTq{i}")
                for i in range(NQ)
            ]
            # sweep-major xT: slice [:, g, :, :] is contiguous per partition
            # (8KB rows) so one dma_start covers a whole column sweep
            xT_r = xw.tile([P, NQ, DC, SCW], bf16)
            wqT_r = xw.tile([P, DC, f], bf16)
            wkT_r = xw.tile([P, DC, f], bf16)
            wvT_r = xw.tile([P, DC, f], bf16)
            woT_sb = xw.tile([P, FC, d], bf16)

            # ones column of V_aug (accumulates softmax denominators in AV)
            nc.vector.memset(V_sb[:, :, :, hd : hd + 1], 1.0)
            nc.vector.memset(V_sb[:, :, :, hd + 1 : hd + 2], 0.0)

            def ktz_zero(g):
                """Zero the sweep-g columns of every head's padded K^T slab
                (just-in-time: only sweep 0 gates the first pass)."""
                nc.vector.memset(
                    KTz_sb[:, :, g * SCW : (g + 1) * SCW], 0.0
                )

            def xsweep(g):
                """xT column sweep g, split across the sync+gpsimd queues.
                Emitted JUST-IN-TIME before its first consumer: tile deps
                batch coarsely, so a consumer emitted after a dma_start
                waits for every earlier writer of that tile."""
                nc.sync.dma_start(
                    out=xT_r[:, g, 0 : DC // 2, :], in_=xT[g][:, 0 : DC // 2, :]
                )
                nc.gpsimd.dma_start(
                    out=xT_r[:, g, DC // 2 :, :], in_=xT[g][:, DC // 2 :, :]
                )

            # first-use-ordered input stream: weights serial on the scalar
            # queue (idle until the first exp ~9us in), sweeps on sync+gpsimd
            nc.scalar.dma_start(out=wqT_r[:], in_=wqT[:])
            xsweep(0)
            nc.sync.dma_start(out=wkT_r[:, 0 : DC // 2, :], in_=wkT[:, 0 : DC // 2, :])
            nc.gpsimd.dma_start(out=wkT_r[:, DC // 2 :, :], in_=wkT[:, DC // 2 :, :])
            nc.scalar.dma_start(out=wvT_r[:], in_=wvT[:])

            def qs(fc, qq):
                """Q^T projection stage: heads 2fc,2fc+1, q columns of
                quarter qq (dc-outer accumulation, one psum bank)."""
                ps = genp.tile([P, SCW], f32, tag="gen", name=f"q{fc}{qq}")
                for dc in range(DC):
                    nc.tensor.matmul(
                        ps[:],
                        wqT_r[:, dc, fc * P : (fc + 1) * P],
                        xT_r[:, qq, dc, :],
                        start=(dc == 0),
                        stop=(dc == DC - 1),
                    )
                nc.vector.tensor_copy(
                    QT_sb[:, fc, qq * SCW : (qq + 1) * SCW], ps[:]
                )

            def ks(fc, g):
                """K^T projection stage: heads 2fc,2fc+1, k columns of
                sweep g; rows land in each head's padded KTz slot."""
                ps = genp.tile([P, SCW], f32, tag="gen", name=f"k{fc}{g}")
                for dc in range(DC):
                    nc.tensor.matmul(
                        ps[:],
                        wkT_r[:, dc, fc * P : (fc + 1) * P],
                        xT_r[:, g, dc, :],
                        start=(dc == 0),
                        stop=(dc == DC - 1),
                    )
                sl = slice(g * SCW, (g + 1) * SCW)
                nc.vector.tensor_copy(
                    KTz_sb[0:hd, 2 * fc, sl], ps[0:hd, :]
                )
                nc.vector.tensor_copy(
                    KTz_sb[hd : 2 * hd, 2 * fc + 1, sl],
                    ps[hd : 2 * hd, :],
                )

            def v_tile(nt):
                """V tile nt in natural [n, feat] layout (stationary = xT
                chunk, moving = wv)."""
                ps = genp.tile([P, SCW], f32, tag="gen", name=f"v{nt}")
                g, j = nt // KG, nt % KG
                for dc in range(DC):
                    nc.tensor.matmul(
                        ps[:, 0:f],
                        xT_r[:, g, dc, j * P : (j + 1) * P],
                        wvT_r[:, dc, :],
                        start=(dc == 0),
                        stop=(dc == DC - 1),
                    )
                nc.vector.tensor_copy(
                    V_sb[:, nt, :, 0:hd],
                    ps[:, 0:f].rearrange("p (h e) -> p h e", h=hpc),
                )

            def pass_begin():
                # pav holds the deferred AV emitter: AV(kc) is emitted after
                # S(kc+1), one exp-period after its pt was written -- an AV
                # issued the moment its exp's semaphore fires reads pt while
                # the ACT's SBUF writes are still committing (+~130ns, seen
                # on nearly every AV of the projection-free quarters)
                return {
                    "avo": avps.tile([hd + 2, SCW], f32, tag="avo", name="avo"),
                    "pav": None,
                }

            def pass_blocks(pd, h, qq, kcs, pre_kc=None, q0=None, w=SCW):
                """scores^T -> exp -> AV accumulate for k-chunks `kcs`.
                q0/w override the q-column window (the final pass runs as
                two 256-wide sub-passes so its tail overlaps wo)."""
                fc = (h * hd) // P
                if q0 is None:
                    q0 = qq * SCW
                avo = pd["avo"]
                kcs = list(kcs)
                # kc PAIRS: one [128, 2, SCW] exp per pair (reads 2 psum
                # banks) halves the ACT instruction count -- ACT is the
                # pacer in the projection-free quarters
                for i in range(0, len(kcs), 2):
                    k0, k1 = kcs[i], kcs[i + 1]
                    sc = scps.tile([P, 2, SCW], f32, tag="sc")
                    for j, kc in ((0, k0), (1, k1)):
                        nc.tensor.matmul(
                            sc[:, j, 0:w],
                            KTz_sb[:, h, kc * P : (kc + 1) * P],
                            QT_sb[:, fc, q0 : q0 + w],
                            start=True,
                            stop=True,
                        )
                    # pre_kc (v_tiles) AFTER the scores pair: a v_tile
                    # waiting on the wv DMA must not block the exp stream
                    if pre_kc is not None:
                        pre_kc(k0)
                        pre_kc(k1)
                    if pd["pav"] is not None:
                        pd["pav"]()
                    pt = (ptp if k0 % 4 == 0 else ptpb).tile(
                        [P, 2, SCW], bf16, tag="pt"
                    )
                    nc.scalar.activation(
                        pt[:, :, 0:w], sc[:, :, 0:w],
                        mybir.ActivationFunctionType.Exp,
                        scale=scale,
                    )

                    def pav(k0=k0, k1=k1, pt=pt):
                        for j, kc in ((0, k0), (1, k1)):
                            nc.tensor.matmul(
                                avo[:, 0:w],
                                V_sb[:, kc, h, :],
                                pt[:, j, 0:w],
                                start=(kc == 0),
                                stop=(kc == NT - 1),
                            )

                    pd["pav"] = pav

            def pass_end(pd, h, qq, q0=None, w=SCW):
                if pd["pav"] is not None:
                    pd["pav"]()
                    pd["pav"] = None
                avo = pd["avo"]
                """Normalize rows 0..hd-1 of avo by row hd (softmax sums):
                approx-reciprocal the PSUM sums row (sums are O(1e3) --
                far from the approximation's edge cases), partition-
                broadcast, multiply straight out of PSUM into outT."""
                fc = (h * hd) // P
                po = (h * hd) % P
                if q0 is None:
                    q0 = qq * SCW
                o0 = q0 - qq * SCW
                sums = normp.tile([1, SCW], f32, tag="sums")
                nc.vector.tensor_copy(
                    sums[:, 0:w], avo[hd : hd + 1, o0 : o0 + w]
                )
                rrow = normp.tile([1, SCW], f32, tag="rrow")
                nc.vector.reciprocal_approx_fast(rrow[:, 0:w], sums[:, 0:w])
                bc = normp.tile([hd, SCW], f32, tag="bc")
                nc.gpsimd.partition_broadcast(bc[:, 0:w], rrow[:, 0:w])
                nc.vector.tensor_mul(
                    outTq[qq][po : po + hd, fc, o0 : o0 + w],
                    avo[0:hd, o0 : o0 + w],
                    bc[:, 0:w],
                )

            # Each pass's normalize chain is EMITTED near the END of the
            # next pass: chains emitted at their natural spot make every
            # later-emitted PE instruction coarse-wait on their mul. The
            # leftover AV flushes early (chunk 0) so its pt buffer frees,
            # while the chain lands before the last 2 kc so only those
            # scores sit behind the (long-since-computed) mul.
            pending_end = [None]

            def flush_av():
                if pending_end[0] is not None:
                    pd = pending_end[0][0]
                    if pd["pav"] is not None:
                        pd["pav"]()
                        pd["pav"] = None

            def flush_end():
                if pending_end[0] is not None:
                    pd, h, qq = pending_end[0]
                    pass_end(pd, h, qq)
                    pending_end[0] = None

            def do_pass(h, qq, pre_kc=None, mid=None):
                avo = pass_begin()
                pass_blocks(avo, h, qq, range(0, KG), pre_kc=pre_kc)
                flush_av()
                pass_blocks(avo, h, qq, range(KG, NT // 2), pre_kc=pre_kc)
                if mid is not None:
                    mid()
                pass_blocks(avo, h, qq, range(NT // 2, NT - 2), pre_kc=pre_kc)
                flush_end()
                pass_blocks(avo, h, qq, range(NT - 2, NT), pre_kc=pre_kc)
                pending_end[0] = (avo, h, qq)

            def wo_blocks(qq, dos, copy_eng="dve", pool=None):
                """Output projection for quarter qq, do-blocks `dos`
                (contract both fc chunks; emits the partial TRANSPOSED
                [d, n]). PSUM->SBUF copies alternate ACT/DVE so neither
                queue backs up ahead of the next quarter's exp/AV chain;
                copy_eng="act" keeps the DVE free (tail filler blocks run
                during the final normalize chain, which lives on DVE)."""
                q0 = qq * SCW
                for do in dos:
                    pl = pool if pool is not None else genp
                    tg = "sc" if pool is not None else "gen"
                    ps = pl.tile([P, SCW], f32, tag=tg, name=f"wo{do}")
                    for fc in range(FC):
                        nc.tensor.matmul(
                            ps[:],
                            woT_sb[:, fc, do * P : (do + 1) * P],
                            outTq[qq][:, fc, :],
                            start=(fc == 0),
                            stop=(fc == FC - 1),
                        )
                    ob = wosbp.tile([P, SCW], bf16, tag="ob")
                    if copy_eng == "act" or (copy_eng == "alt" and do % 2 == 0):
                        nc.scalar.activation(
                            ob[:], ps[:], mybir.ActivationFunctionType.Copy
                        )
                    else:
                        nc.vector.tensor_copy(ob[:], ps[:])
                    # sync-queue descriptors process ~4x faster than
                    # gpsimd-queue ones (42ns vs 155ns each, measured), and
                    # the input stream is done by the time wo runs
                    nc.sync.dma_start(
                        out=out[do * P : (do + 1) * P, q0 : q0 + SCW],
                        in_=ob[:],
                    )

            def wo_q3_fc0(dos):
                """The fc0 (heads 0,1) half of quarter 3's output
                projection, emitted inside pass(2,3) as soon as those heads
                are final. Lands in out2, summed on the host."""
                for do in dos:
                    ps = genp.tile([P, SCW], f32, tag="gen", name=f"w3a{do}")
                    nc.tensor.matmul(
                        ps[:],
                        woT_sb[:, 0, do * P : (do + 1) * P],
                        outTq[3][:, 0, :],
                        start=True,
                        stop=True,
                    )
                    ob = wosbp.tile([P, SCW], bf16, tag="ob")
                    nc.vector.tensor_copy(ob[:], ps[:])
                    nc.sync.dma_start(
                        out=out2[do * P : (do + 1) * P, :], in_=ob[:]
                    )

            # persistent tail output staging: fc1 halves land per-do, one
            # full-width DMA per do after its second half
            obq3 = outp.tile([P, d // P, SCW], bf16)

            def wo_q3_fc1(half, dos):
                """Tail: fc1-only wo for one 256-wide half of quarter 3
                (half A runs while half B's normalize chain is still in
                flight). Copies stay on ACT for half A (DVE owns the norm
                chains), alternate for half B."""
                q3 = 3 * SCW
                o0 = half * 256
                for do in dos:
                    ps = genp.tile([P, SCW], f32, tag="gen", name=f"w3b{do}")
                    nc.tensor.matmul(
                        ps[:, 0:256],
                        woT_sb[:, 1, do * P : (do + 1) * P],
                        outTq[3][:, 1, o0 : o0 + 256],
                        start=True,
                        stop=True,
                    )
                    if do % 2 == 0:
                        nc.scalar.activation(
                            obq3[:, do, o0 : o0 + 256], ps[:, 0:256],
                            mybir.ActivationFunctionType.Copy,
                        )
                    else:
                        nc.vector.tensor_copy(
                            obq3[:, do, o0 : o0 + 256], ps[:, 0:256]
                        )
                    if half == 1:
                        eng = nc.sync if do % 2 == 0 else nc.gpsimd
                        eng.dma_start(
                            out=out[do * P : (do + 1) * P, q3 : q3 + SCW],
                            in_=obq3[:, do, :],
                        )

            # ---- emission order = scheduling priority ----
            # quarter 0: the xT DMA stream is the gate; interleave the h0
            # and h1 passes sweep-by-sweep so every landed sweep unlocks
            # ~2x the PE work (both heads' scores + the fc1 projections
            # that only need sweep 0)
            ktz_zero(0)
            qs(0, 0)
            ks(0, 0)
            avo0 = pass_begin()
            pass_blocks(avo0, 0, 0, range(0, KG), pre_kc=v_tile)
            ks(1, 0)
            qs(1, 0)
            avo1 = pass_begin()
            pass_blocks(avo1, 1, 0, range(0, KG))
            xsweep(1)
            ktz_zero(1)
            ks(0, 1)
            pass_blocks(avo0, 0, 0, range(KG, 2 * KG), pre_kc=v_tile)
            ks(1, 1)
            pass_blocks(avo1, 1, 0, range(KG, 2 * KG))
            xsweep(2)
            ktz_zero(2)
            ks(0, 2)
            pass_blocks(avo0, 0, 0, range(2 * KG, 3 * KG), pre_kc=v_tile)
            ks(1, 2)
            pass_blocks(avo1, 1, 0, range(2 * KG, 3 * KG))
            xsweep(3)
            ktz_zero(3)
            ks(0, 3)
            pass_blocks(avo0, 0, 0, range(3 * KG, NT), pre_kc=v_tile)
            ks(1, 3)
            nc.sync.dma_start(out=woT_sb[:], in_=woT[:])
            pass_blocks(avo1, 1, 0, range(3 * KG, NT))
            pass_end(avo0, 0, 0)
            pending_end[0] = (avo1, 1, 0)

            do_pass(2, 0, mid=lambda: qs(0, 1))
            do_pass(3, 0, mid=lambda: qs(1, 1))
            # wo for a finished quarter is spread through the next quarter's
            # passes via the mid hook: cross-engine waits batch coarsely
            # (anything emitted after a pass_end waits on its mul), so the
            # blocks must be emitted BEFORE the surrounding pass_end
            def mids(*fns):
                return lambda: [fn() for fn in fns]

            do_pass(0, 1, mid=lambda: qs(0, 2))
            do_pass(1, 1, mid=lambda: qs(1, 2))
            do_pass(2, 1, mid=lambda: qs(0, 3))
            do_pass(3, 1, mid=lambda: qs(1, 3))
            for h in range(hpc):
                do_pass(h, 2, mid=lambda h=h: wo_blocks(0, [2 * h, 2 * h + 1]))
            do_pass(0, 3, mid=lambda: wo_blocks(1, [0, 1, 2, 3]))
            do_pass(1, 3, mid=lambda: wo_blocks(1, [4, 5, 6, 7]))
            # fc0's out2 stream rides pass(2,3): heads 0,1 are done, and
            # emitting it here lets its 1MB of output DMA drain during
            # compute instead of stacking onto the post-kernel drain
            do_pass(
                2, 3,
                mid=mids(
                    lambda: wo_blocks(2, [0, 1, 2, 3]),
                    lambda: wo_q3_fc0(range(8)),
                ),
            )
            # final pass. The tail is software-pipelined: the normalize is
            # split into two half-width chains, fc0 filler blocks keep the
            # PE warm through chain A, fc1's half-A wo overlaps chain B.
            q3 = 3 * SCW
            avoz = pass_begin()
            pass_blocks(avoz, 3, 3, range(0, KG))
            flush_av()
            pass_blocks(avoz, 3, 3, range(KG, NT // 2))
            wo_blocks(2, [4, 5], copy_eng="dve")
            pass_blocks(avoz, 3, 3, range(NT // 2, NT - 2))
            flush_end()
            pass_blocks(avoz, 3, 3, range(NT - 2, NT))
            wo_blocks(2, [6, 7], copy_eng="dve")
            pass_end(avoz, 3, 3, q0=q3, w=256)
            wo_q3_fc1(0, range(8))
            pass_end(avoz, 3, 3, q0=q3 + 256, w=256)
            wo_q3_fc1(1, range(8))
    nc.finalize()
    return nc


def make_in_maps(x, Wq, Wk, Wv, Wo):
    """Shard full inputs into per-core DRAM parameter maps (bf16)."""
    import ml_dtypes

    bf16 = ml_dtypes.bfloat16
    DC, NQ, SCW = D // P, N // 512, 512

    def w_pre(wT):  # [d_or_f, cols] -> [P, chunks, cols]
        return np.ascontiguousarray(
            wT.reshape(-1, P, wT.shape[1]).transpose(1, 0, 2)
        ).astype(bf16)

    x = np.asarray(x, dtype=np.float32)
    # [d, n] -> sweep-major [NQ, P, DC, SCW] matching the xT_r tile
    xTs = [
        np.ascontiguousarray(
            x[b].T.reshape(DC, P, NQ, SCW).transpose(2, 1, 0, 3)
        ).astype(bf16)
        for b in range(B)
    ]
    WqT = np.asarray(Wq, np.float32).T
    WkT = np.asarray(Wk, np.float32).T
    WvT = np.asarray(Wv, np.float32).T
    WoB = np.asarray(Wo, np.float32)
    in_maps = []
    for c in range(N_CORES):
        b, g = c // (N_CORES // B), c % (N_CORES // B)
        fs = slice(g * F, (g + 1) * F)
        in_maps.append(
            {
                "xT": xTs[b],
                "wqT": w_pre(WqT[:, fs]),
                "wkT": w_pre(WkT[:, fs]),
                "wvT": w_pre(WvT[:, fs]),
                "woT": w_pre(np.ascontiguousarray(WoB[:, fs].T)),
            }
        )
    return in_maps


_NC_CACHE = {}


def run(x, Wq, Wk, Wv, Wo, trace=False):
    from concourse.bass_utils import run_bass_kernel_spmd

    # NOTE: walrus --enable-ldw-opt stays at its default (false): v2 has no
    # consecutive matmuls sharing a stationary (SCW == QB == 512), and the
    # bf16 Ldweights form is rejected by the opt's codegen path anyway.
    if "nc" not in _NC_CACHE:
        _NC_CACHE["nc"] = build_nc()
    nc = _NC_CACHE["nc"]
    in_maps = make_in_maps(x, Wq, Wk, Wv, Wo)
    res = run_bass_kernel_spmd(nc, in_maps, core_ids=list(range(N_CORES)), trace=trace)
    parts = []
    for i in range(N_CORES):
        p = np.asarray(res.results[i]["out"]).astype(np.float32)
        # quarter 3's output was emitted in two halves: fc1 went to out,
        # fc0 to out2 -- recombine here
        p[:, 3 * 512 :] += np.asarray(res.results[i]["out2"]).astype(
            np.float32
        )
        parts.append(p)
    gpb = N_CORES // B
    # per-core partials are transposed [d, n]: sum the group, then untranspose
    full = np.stack(
        [
            sum(parts[b * gpb + 1 : (b + 1) * gpb], parts[b * gpb]).T
            for b in range(B)
        ]
    )
    return np.ascontiguousarray(full, dtype=np.float32), res


def kernel(x, Wq, bq, Wk, bk, Wv, bv, Wo, bo):
    full, _ = run(x, Wq, Wk, Wv, Wo)
    return full
